# revision 29
# baseline (speedup 1.0000x reference)
"""Trainium2 Bass kernel for a 2-layer LSTM decoder (autoregressive, 512 steps).

Strategy (data-parallel over batch):
  - BATCH=1024 split as 128 rows per core across 8 cores; the 128 batch rows
    sit on the SBUF/PSUM partition dimension.
  - Matmuls are state-stationary fp16: out[batch, gates] = lhsT.T @ rhs with
    lhsT = transposed state tiles [K=h/x dims, M=128 batch] and rhs = host-
    pre-transposed weights [K, 4H] streamed N=512 per PSUM bank.
  - Single ACT table set (exp_and_others): sigmoid(x) computed as
    0.5*tanh(x/2)+0.5 with the 0.5 gate-scales folded into the weights and the
    (s+1) affines folded into fused scalar_tensor_tensor ops; h-state kept
    doubled (H=2h) with h-side weights pre-halved so no extra scaling ops.
  - log-sum-exp for log_softmax: exp with ACT accum_out gives s = sum(exp(p));
    ln(s) via exponent bit-hack initial guess + one Newton step (uses exp,
    stays in the same ACT table set).
  - The -lse and sigmoid(p63) contributions to the next step's input are
    applied as a K=2 rank-1 matmul so they stay off the critical path.
"""

import math
import sys

import numpy as np

if "/opt/trn_rl_repo" not in sys.path:
    sys.path.insert(0, "/opt/trn_rl_repo")

import concourse.bass as bass
import concourse.mybir as mybir
import concourse.tile as tile
from concourse.bass_utils import run_bass_kernel_spmd
from concourse.masks import make_identity

F16 = mybir.dt.float16
F32 = mybir.dt.float32
I32 = mybir.dt.int32
AF = mybir.ActivationFunctionType
OP = mybir.AluOpType

INPUT_SIZE = 64
HIDDEN = 256
OUTPUT_SIZE = 64
MAX_LENGTH = 512
BATCH = 1024
NCORES = 8
BLOC = BATCH // NCORES  # 128
G4 = 4 * HIDDEN  # 1024

# ln(x) ~= A*float(bitcast_i32(x)) + B, then one Newton step y += x*exp(-y)-1
LN2 = 0.6931471805599453
LN_A = LN2 / (1 << 23)
LN_B = LN2 * (0.0430357 - 127.0)

# gate permutation: torch order rows [i f g o] each 256 -> chunked layout
# [i_a f_a g_a o_a | i_b f_b g_b o_b] with a = h-dims 0:128, b = 128:256
_PERM = np.concatenate(
    [
        np.r_[0:128], np.r_[512:640], np.r_[256:384], np.r_[768:896],
        np.r_[128:256], np.r_[640:768], np.r_[384:512], np.r_[896:1024],
    ]
)
# gate scale: 0.5 for i,f,o (sigmoid-via-tanh), 1.0 for g (plain tanh)
_GS = np.ones(G4, np.float32)
_GS[0:256] = 0.5
_GS[256:512] = 0.5
_GS[768:1024] = 0.5
_GS_PERM = _GS[_PERM]


def _chunk_cols(c):
    return slice(c * 512, (c + 1) * 512)


def build_bass(steps=MAX_LENGTH, timing_ring=False):
    """Build the single-core Bass program (identical across cores).

    timing_ring=True shrinks the outputs tensor to a 16-step ring so timing
    runs avoid the large host<->device transfer (outputs then invalid).
    """
    from contextlib import ExitStack

    nc = bass.Bass("TRN2", target_bir_lowering=False, debug=False)

    # --- DRAM I/O ----------------------------------------------------------
    d_ih0 = nc.dram_tensor("w_ih0", [INPUT_SIZE + 2, G4], F16, kind="ExternalInput").ap()
    d_hh0a = nc.dram_tensor("w_hh0a", [128, G4], F16, kind="ExternalInput").ap()
    d_hh0b = nc.dram_tensor("w_hh0b", [128, G4], F16, kind="ExternalInput").ap()
    d_ih1a = nc.dram_tensor("w_ih1a", [128, G4], F16, kind="ExternalInput").ap()
    d_ih1b = nc.dram_tensor("w_ih1b", [128, G4], F16, kind="ExternalInput").ap()
    d_hh1a = nc.dram_tensor("w_hh1a", [128, G4], F16, kind="ExternalInput").ap()
    d_hh1b = nc.dram_tensor("w_hh1b", [128, G4], F16, kind="ExternalInput").ap()
    d_fca = nc.dram_tensor("w_fca", [128, OUTPUT_SIZE], F16, kind="ExternalInput").ap()
    d_fcb = nc.dram_tensor("w_fcb", [128, OUTPUT_SIZE], F16, kind="ExternalInput").ap()
    d_b1 = nc.dram_tensor("b1row", [1, G4], F16, kind="ExternalInput").ap()
    d_bfc = nc.dram_tensor("bfcrow", [1, OUTPUT_SIZE], F16, kind="ExternalInput").ap()
    d_h0t = nc.dram_tensor("h0t_init", [128, HIDDEN], F16, kind="ExternalInput").ap()
    d_h1t = nc.dram_tensor("h1t_init", [128, HIDDEN], F16, kind="ExternalInput").ap()
    d_ct0 = nc.dram_tensor("ct0_init", [BLOC, HIDDEN], F32, kind="ExternalInput").ap()
    d_ct1 = nc.dram_tensor("ct1_init", [BLOC, HIDDEN], F32, kind="ExternalInput").ap()
    d_xt = nc.dram_tensor("xt_init", [INPUT_SIZE + 2, BLOC], F16, kind="ExternalInput").ap()

    d_outs = nc.dram_tensor(
        "outs", [BLOC, 16 if timing_ring else steps, INPUT_SIZE + 2], F16,
        kind="ExternalOutput",
    ).ap()
    d_hf = nc.dram_tensor("hf", [2, BLOC, HIDDEN], F16, kind="ExternalOutput").ap()
    d_cf = nc.dram_tensor("cf", [2, BLOC, HIDDEN], F32, kind="ExternalOutput").ap()

    with ExitStack() as ctx:
        tc = ctx.enter_context(tile.TileContext(nc))
        singles = ctx.enter_context(tc.tile_pool(name="singles", bufs=1))
        work = ctx.enter_context(tc.tile_pool(name="work", bufs=3))
        gpsum = ctx.enter_context(tc.tile_pool(name="gpsum", bufs=1, space="PSUM"))
        ppsum = ctx.enter_context(tc.tile_pool(name="ppsum", bufs=1, space="PSUM"))
        tpsum = ctx.enter_context(tc.tile_pool(name="tpsum", bufs=3, space="PSUM"))

        # --- persistent SBUF ----------------------------------------------
        w_ih0 = singles.tile([INPUT_SIZE + 2, G4], F16)
        w_hh0 = [singles.tile([128, G4], F16, tag=f"whh0{c}", name=f"whh0{c}") for c in range(2)]
        w_ih1 = [singles.tile([128, G4], F16, tag=f"wih1{c}", name=f"wih1{c}") for c in range(2)]
        w_hh1 = [singles.tile([128, G4], F16, tag=f"whh1{c}", name=f"whh1{c}") for c in range(2)]
        w_fc = [singles.tile([128, OUTPUT_SIZE], F16, tag=f"wfc{c}", name=f"wfc{c}") for c in range(2)]
        b1row = singles.tile([1, G4], F16)
        bfcrow = singles.tile([1, OUTPUT_SIZE], F16)
        ident = singles.tile([128, 128], F16)
        ones1 = singles.tile([1, 128], F16)
        xsb_pp = [
            singles.tile([128, INPUT_SIZE + 2], F16, tag=f"xsb{i}", name=f"xsb{i}")
            for i in range(2)
        ]
        xT = singles.tile([INPUT_SIZE + 2, BLOC], F16)
        h0T = singles.tile([128, HIDDEN], F16)
        h1T = singles.tile([128, HIDDEN], F16)
        statep = ctx.enter_context(tc.tile_pool(name="statep", bufs=3))

        nc.sync.dma_start(w_ih0, d_ih0)
        nc.sync.dma_start(w_hh0[0], d_hh0a)
        nc.sync.dma_start(w_hh0[1], d_hh0b)
        nc.sync.dma_start(w_ih1[0], d_ih1a)
        nc.sync.dma_start(w_ih1[1], d_ih1b)
        nc.sync.dma_start(w_hh1[0], d_hh1a)
        nc.sync.dma_start(w_hh1[1], d_hh1b)
        nc.sync.dma_start(w_fc[0], d_fca)
        nc.sync.dma_start(w_fc[1], d_fcb)
        nc.sync.dma_start(b1row, d_b1)
        nc.sync.dma_start(bfcrow, d_bfc)
        nc.sync.dma_start(h0T, d_h0t)
        nc.sync.dma_start(h1T, d_h1t)
        nc.sync.dma_start(xT, d_xt)

        make_identity(nc, ident)
        nc.vector.memset(ones1, 1.0)
        for i in range(2):
            nc.vector.memset(xsb_pp[i][:, 0:66], 0.0)
            nc.vector.memset(xsb_pp[i][:, 65:66], 1.0)
            nc.vector.memset(xsb_pp[i][:, 61:62], 1.0)  # step-0 one-hot (overwritten later)
            nc.vector.memset(xsb_pp[i][:, 63:64], -1.0)

        ct = [singles.tile([BLOC, HIDDEN], F32, tag=f"ct_init{l}", name=f"ct_init{l}") for l in range(2)]
        nc.sync.dma_start(ct[0], d_ct0)
        nc.sync.dma_start(ct[1], d_ct1)

        hT = [h0T, h1T]
        w_hh = [w_hh0, w_hh1]
        Hnew_last = [None, None]

        def emit_hh(g, layer, first):
            """hh (and for t=0 only) matmuls opening each chunk's accum group."""
            for c in range(2):
                cs = _chunk_cols(c)
                nc.tensor.matmul(g[:, cs], hT[layer][:, 0:128], w_hh[layer][0][:, cs], start=True, stop=False)
                nc.tensor.matmul(g[:, cs], hT[layer][:, 128:256], w_hh[layer][1][:, cs], start=False, stop=False)

        def emit_cell(g, layer, t):
            """tanh + cell update for one layer; returns nothing (updates hT/ct)."""
            tall = work.tile([BLOC, G4], F16, tag=f"tall{layer}", name=f"tall{layer}")
            ct_new = work.tile([BLOC, HIDDEN], F32, tag=f"ct{layer}", name=f"ct{layer}")
            Hnew = work.tile([BLOC, HIDDEN], F16, tag=f"H{layer}", name=f"H{layer}")
            hT_new = statep.tile([128, HIDDEN], F16, tag=f"hT{layer}", name=f"hT{layer}")
            Hnew_last[layer] = Hnew
            sig = {}
            for c in range(2):
                o0 = c * 512
                nc.scalar.activation(tall[:, o0 : o0 + 512], g[:, o0 : o0 + 512], AF.Tanh)
                si = tall[:, o0 : o0 + 128]
                tg = tall[:, o0 + 128 : o0 + 256]
                sf = tall[:, o0 + 256 : o0 + 384]
                hs = slice(c * 128, (c + 1) * 128)
                A = work.tile([BLOC, 128], F16, tag=f"A{layer}{c}", name=f"A{layer}{c}")
                B = work.tile([BLOC, 128], F32, tag=f"B{layer}{c}", name=f"B{layer}{c}")
                tcn = work.tile([BLOC, 128], F16, tag=f"tc{layer}{c}", name=f"tc{layer}{c}")
                # state is D = 2c:  A = 2*sig_i*tanh(g);  B = 2*sig_f*D
                # D_new = 2*c_new = 0.5*B + A;  tanh(c_new) = tanh(0.5*D_new)
                nc.vector.scalar_tensor_tensor(A, si, 1.0, tg, OP.add, OP.mult)
                nc.vector.scalar_tensor_tensor(B, sf, 1.0, ct[layer][:, hs], OP.add, OP.mult)
                nc.vector.scalar_tensor_tensor(ct_new[:, hs], B, 0.5, A, OP.mult, OP.add)
                nc.scalar.activation(tcn, ct_new[:, hs], AF.Tanh, scale=0.5)
                sig[c] = tcn
            for c in range(2):
                o0 = c * 512
                so = tall[:, o0 + 384 : o0 + 512]
                hs = slice(c * 128, (c + 1) * 128)
                nc.vector.scalar_tensor_tensor(Hnew[:, hs], so, 1.0, sig[c], OP.add, OP.mult)
                tp = tpsum.tile([128, 128], F16, tag="hT", name="tp")
                nc.tensor.transpose(tp, Hnew[:, hs], ident)
                nc.vector.tensor_copy(hT_new[:, hs], tp)
            ct[layer] = ct_new
            hT[layer] = hT_new

        # prologue: open gates0 accumulation for step 0 with the hh matmuls
        g0 = gpsum.tile([BLOC, G4], F32, tag="g0", name="g0")
        emit_hh(g0, 0, True)

        for t in range(steps):
            xsb = xsb_pp[(t + 1) % 2]  # tile being written this step (for step t+1)

            # close gates0: input matmuls (x'' includes tanh/lse/bias columns)
            for c in range(2):
                cs = _chunk_cols(c)
                nc.tensor.matmul(g0[:, cs], xT, w_ih0[:, cs], start=False, stop=True)

            # open gates1 with its hh + bias matmuls (ready at step start)
            g1 = gpsum.tile([BLOC, G4], F32, tag="g1", name="g1")
            emit_hh(g1, 1, False)
            for c in range(2):
                cs = _chunk_cols(c)
                nc.tensor.matmul(g1[:, cs], ones1, b1row[:, cs], start=False, stop=False)

            # layer-0 cell
            emit_cell(g0, 0, t)

            # close gates1 (k-tile-a work first so nothing head-of-line blocks)
            if t != steps - 1:
                g0n = gpsum.tile([BLOC, G4], F32, tag="g0", name="g0")
            else:
                g0n = None
            for c in range(2):
                cs = _chunk_cols(c)
                nc.tensor.matmul(g1[:, cs], hT[0][:, 0:128], w_ih1[0][:, cs], start=False, stop=False)
            if g0n is not None:
                for c in range(2):
                    cs = _chunk_cols(c)
                    nc.tensor.matmul(g0n[:, cs], hT[0][:, 0:128], w_hh0[0][:, cs], start=True, stop=False)
            for c in range(2):
                cs = _chunk_cols(c)
                nc.tensor.matmul(g1[:, cs], hT[0][:, 128:256], w_ih1[1][:, cs], start=False, stop=True)
            if g0n is not None:
                for c in range(2):
                    cs = _chunk_cols(c)
                    nc.tensor.matmul(g0n[:, cs], hT[0][:, 128:256], w_hh0[1][:, cs], start=False, stop=False)

            # layer-1 cell
            emit_cell(g1, 1, t)

            # FC
            pp = ppsum.tile([BLOC, OUTPUT_SIZE], F32, tag="pp", name="pp")
            nc.tensor.matmul(pp, hT[1][:, 0:128], w_fc[0], start=True, stop=False)
            nc.tensor.matmul(pp, hT[1][:, 128:256], w_fc[1], start=False, stop=False)
            nc.tensor.matmul(pp, ones1, bfcrow, start=False, stop=True)

            # softmax pieces: s = sum(exp(p)), tp-col, lse via bit-hack+Newton
            e63 = work.tile([BLOC, 63], F32, tag="e63", name="e63")
            ssum = work.tile([BLOC, 1], F32, tag="ssum", name="ssum")
            nc.scalar.activation(e63, pp[:, 0:63], AF.Exp, accum_out=ssum)
            nc.scalar.activation(xsb[:, 63:64], pp[:, 63:64], AF.Tanh, scale=0.5)
            conv = work.tile([BLOC, 1], F32, tag="conv", name="conv")
            ny0 = work.tile([BLOC, 1], F32, tag="ny0", name="ny0")
            en = work.tile([BLOC, 1], F32, tag="en", name="en")
            u = work.tile([BLOC, 1], F32, tag="u", name="u")
            nlse = work.tile([BLOC, 1], F32, tag="nlse", name="nlse")
            nc.vector.tensor_copy(conv, ssum.bitcast(I32))
            nc.vector.tensor_scalar(ny0, conv, -LN_A, -LN_B, OP.mult, OP.add)
            nc.scalar.activation(en, ny0, AF.Exp)
            nc.vector.tensor_copy(xsb[:, 0:63], pp[:, 0:63])
            nc.vector.tensor_tensor(u, ssum, en, OP.mult)
            nc.vector.scalar_tensor_tensor(nlse, ny0, 1.0, u, OP.add, OP.subtract)

            # next-step input columns + transpose (ring-critical)
            nc.vector.tensor_copy(xsb[:, 64:65], nlse)
            if t != steps - 1:
                xT_new = statep.tile([INPUT_SIZE + 2, BLOC], F16, tag="xT", name="xT")
                tpx = tpsum.tile([INPUT_SIZE + 2, BLOC], F16, tag="hT", name="tpx")
                nc.tensor.transpose(tpx, xsb, ident)
                nc.vector.tensor_copy(xT_new, tpx)
                xT = xT_new

            # raw per-step output snapshot: host derives log_softmax + dur
            nc.sync.dma_start(
                d_outs[:, (t % 16 if timing_ring else t), :], xsb[:, :]
            )
            g0 = g0n

        # ---------------- final states -----------------------------------
        nc.sync.dma_start(d_hf[0], Hnew_last[0])
        nc.sync.dma_start(d_hf[1], Hnew_last[1])
        nc.sync.dma_start(d_cf[0], ct[0])
        nc.sync.dma_start(d_cf[1], ct[1])

    return nc


def _legalize_waits(nc):
    """walrus accepts at most one sync-wait per compute/DMA instruction.
    Hoist excess waits onto same-engine NoOps inserted just before."""
    uid = 0
    for bb in nc.m.functions[0].blocks:
        il = bb.instructions
        if not any(
            getattr(i, "sync_info", None)
            and i.sync_info.on_wait
            and len(i.sync_info.on_wait) > 1
            for i in il
        ):
            continue
        new = []
        for inst in il:
            si = getattr(inst, "sync_info", None)
            if si and si.on_wait and len(si.on_wait) > 1:
                waits = list(si.on_wait)
                for w in waits[:-1]:
                    nop = mybir.InstEventSemaphore(name=f"waitnop-{uid}", ins=[], outs=[])
                    uid += 1
                    nop.engine = inst.engine
                    nop.sync_info = mybir.SyncInfo(on_wait=[w], on_update=[])
                    new.append(nop)
                inst.sync_info = mybir.SyncInfo(
                    on_wait=[waits[-1]], on_update=si.on_update
                )
            new.append(inst)
        bb.instructions = new


def prep_inputs(inputs, steps=MAX_LENGTH):
    """Host-side prep: returns per-core in_maps for run_bass_kernel_spmd."""
    f = np.asarray
    Wih0 = f(inputs["Wih0"], np.float32)
    Whh0 = f(inputs["Whh0"], np.float32)
    Wih1 = f(inputs["Wih1"], np.float32)
    Whh1 = f(inputs["Whh1"], np.float32)
    Wfc = f(inputs["Wfc"], np.float32)
    bih0 = f(inputs["bih0"], np.float32)
    bhh0 = f(inputs["bhh0"], np.float32)
    bih1 = f(inputs["bih1"], np.float32)
    bhh1 = f(inputs["bhh1"], np.float32)
    bfc = f(inputs["bfc"], np.float32)
    h0 = f(inputs["h0"], np.float32)
    c0 = f(inputs["c0"], np.float32)

    gsp = _GS_PERM[:, None]
    V0 = (Wih0 * _GS[:, None])[_PERM]  # [1024, 64] permuted+scaled
    b0 = ((bih0 + bhh0) * _GS)[_PERM]

    # x'' columns: [p(63) | tanh(p63/2) | -lse | 1]
    w_ih0 = np.zeros((INPUT_SIZE + 2, G4), np.float16)
    w_ih0[0:63] = V0[:, 0:63].T
    w_ih0[63] = 0.5 * V0[:, 63]
    w_ih0[64] = V0[:, 0:63].sum(1)
    w_ih0[65] = b0 + 0.5 * V0[:, 63]

    whh0 = (Whh0[_PERM] * gsp * 0.5).T.astype(np.float16)  # [256, 1024]
    wih1 = (Wih1[_PERM] * gsp * 0.5).T.astype(np.float16)
    whh1 = (Whh1[_PERM] * gsp * 0.5).T.astype(np.float16)
    wfc = (Wfc * 0.5).T.astype(np.float16)  # [256, 64]
    b1row = ((bih1 + bhh1) * _GS)[_PERM][None].astype(np.float16)
    bfcrow = bfc[None].astype(np.float16)

    xt_init = np.zeros((INPUT_SIZE + 2, BLOC), np.float16)
    xt_init[61] = 1.0   # SOS one-hot
    xt_init[63] = -1.0  # tanh col: 0.5*(-1)+0.5 = 0 = x0[63]
    xt_init[65] = 1.0   # const-1 bias row

    shared = dict(
        w_ih0=w_ih0,
        w_hh0a=whh0[0:128].copy(), w_hh0b=whh0[128:256].copy(),
        w_ih1a=wih1[0:128].copy(), w_ih1b=wih1[128:256].copy(),
        w_hh1a=whh1[0:128].copy(), w_hh1b=whh1[128:256].copy(),
        w_fca=wfc[0:128].copy(), w_fcb=wfc[128:256].copy(),
        b1row=b1row, bfcrow=bfcrow,
        xt_init=xt_init,
    )

    in_maps = []
    for i in range(NCORES):
        bsl = slice(i * BLOC, (i + 1) * BLOC)
        H0 = 2.0 * h0[0][bsl]  # [128, 256]
        H1 = 2.0 * h0[1][bsl]
        m = dict(shared)
        m["h0t_init"] = np.concatenate([H0[:, 0:128].T, H0[:, 128:256].T], 1).astype(np.float16)
        m["h1t_init"] = np.concatenate([H1[:, 0:128].T, H1[:, 128:256].T], 1).astype(np.float16)
        m["ct0_init"] = (2.0 * c0[0][bsl]).astype(np.float32)
        m["ct1_init"] = (2.0 * c0[1][bsl]).astype(np.float32)
        in_maps.append(m)
    return in_maps


_BUILD_CACHE = {}


def get_built(steps=MAX_LENGTH):
    if steps not in _BUILD_CACHE:
        _BUILD_CACHE[steps] = build_bass(steps)
    return _BUILD_CACHE[steps]


def run(inputs, steps=MAX_LENGTH, trace=False, **kw):
    nc = get_built(steps)
    if not getattr(nc, "_waits_legalized", False):
        _legalize_waits(nc)
        nc._waits_legalized = True
    in_maps = prep_inputs(inputs, steps)
    res = run_bass_kernel_spmd(nc, in_maps, core_ids=list(range(NCORES)), trace=trace, **kw)
    raw = np.concatenate([r["outs"] for r in res.results], 0).astype(np.float32)
    # raw[:, t] = [p(63) | tanh(p63/2) | -lse | 1]
    outputs = np.empty((BATCH, MAX_LENGTH, OUTPUT_SIZE), np.float32)
    outputs[:, :, 0:63] = raw[:, :, 0:63] + raw[:, :, 64:65]
    outputs[:, :, 63] = 0.5 * raw[:, :, 63] + 0.5
    h_f = np.stack(
        [
            np.concatenate([r["hf"][0].astype(np.float32) for r in res.results], 0),
            np.concatenate([r["hf"][1].astype(np.float32) for r in res.results], 0),
        ]
    ) * 0.5
    c_f = np.stack(
        [
            np.concatenate([r["cf"][0] for r in res.results], 0),
            np.concatenate([r["cf"][1] for r in res.results], 0),
        ]
    ) * 0.5
    return (outputs, h_f, c_f), res


def kernel(**inputs):
    (outputs, h_f, c_f), _ = run(inputs)
    return outputs, h_f, c_f


# revision 30
# speedup vs baseline: 1.0195x; 1.0195x over previous
"""Trainium2 Bass kernel for a 2-layer LSTM decoder (autoregressive, 512 steps).

Strategy (data-parallel over batch):
  - BATCH=1024 split as 128 rows per core across 8 cores; the 128 batch rows
    sit on the SBUF/PSUM partition dimension.
  - Matmuls are state-stationary fp16: out[batch, gates] = lhsT.T @ rhs with
    lhsT = transposed state tiles [K=h/x dims, M=128 batch] and rhs = host-
    pre-transposed weights [K, 4H] streamed N=512 per PSUM bank.
  - Single ACT table set (exp_and_others): sigmoid(x) computed as
    0.5*tanh(x/2)+0.5 with the 0.5 gate-scales folded into the weights and the
    (s+1) affines folded into fused scalar_tensor_tensor ops; h-state kept
    doubled (H=2h) with h-side weights pre-halved so no extra scaling ops.
  - log-sum-exp for log_softmax: exp with ACT accum_out gives s = sum(exp(p));
    ln(s) via exponent bit-hack initial guess + one Newton step (uses exp,
    stays in the same ACT table set).
  - The -lse and sigmoid(p63) contributions to the next step's input are
    applied as a K=2 rank-1 matmul so they stay off the critical path.
"""

import math
import sys

import numpy as np

if "/opt/trn_rl_repo" not in sys.path:
    sys.path.insert(0, "/opt/trn_rl_repo")

import concourse.bass as bass
import concourse.mybir as mybir
import concourse.tile as tile
from concourse.bass_utils import run_bass_kernel_spmd
from concourse.masks import make_identity

F16 = mybir.dt.float16
F32 = mybir.dt.float32
I32 = mybir.dt.int32
AF = mybir.ActivationFunctionType
OP = mybir.AluOpType

INPUT_SIZE = 64
HIDDEN = 256
OUTPUT_SIZE = 64
MAX_LENGTH = 512
BATCH = 1024
NCORES = 8
BLOC = BATCH // NCORES  # 128
G4 = 4 * HIDDEN  # 1024

# ln(x) ~= A*float(bitcast_i32(x)) + B, then one Newton step y += x*exp(-y)-1
LN2 = 0.6931471805599453
LN_A = LN2 / (1 << 23)
LN_B = LN2 * (0.0430357 - 127.0)

# gate permutation: torch order rows [i f g o] each 256 -> chunked layout
# [i_a f_a g_a o_a | i_b f_b g_b o_b] with a = h-dims 0:128, b = 128:256
_PERM = np.concatenate(
    [
        np.r_[0:128], np.r_[512:640], np.r_[256:384], np.r_[768:896],
        np.r_[128:256], np.r_[640:768], np.r_[384:512], np.r_[896:1024],
    ]
)
# gate scale: 0.5 for i,f,o (sigmoid-via-tanh), 1.0 for g (plain tanh)
_GS = np.ones(G4, np.float32)
_GS[0:256] = 0.5
_GS[256:512] = 0.5
_GS[768:1024] = 0.5
_GS_PERM = _GS[_PERM]


def _chunk_cols(c):
    return slice(c * 512, (c + 1) * 512)


def build_bass(steps=MAX_LENGTH, timing_ring=False):
    """Build the single-core Bass program (identical across cores).

    timing_ring=True shrinks the outputs tensor to a 16-step ring so timing
    runs avoid the large host<->device transfer (outputs then invalid).
    """
    from contextlib import ExitStack

    nc = bass.Bass("TRN2", target_bir_lowering=False, debug=False)

    # --- DRAM I/O ----------------------------------------------------------
    d_ih0 = nc.dram_tensor("w_ih0", [INPUT_SIZE + 2, G4], F16, kind="ExternalInput").ap()
    d_hh0a = nc.dram_tensor("w_hh0a", [128, G4], F16, kind="ExternalInput").ap()
    d_hh0b = nc.dram_tensor("w_hh0b", [128, G4], F16, kind="ExternalInput").ap()
    d_ih1a = nc.dram_tensor("w_ih1a", [128, G4], F16, kind="ExternalInput").ap()
    d_ih1b = nc.dram_tensor("w_ih1b", [128, G4], F16, kind="ExternalInput").ap()
    d_hh1a = nc.dram_tensor("w_hh1a", [128, G4], F16, kind="ExternalInput").ap()
    d_hh1b = nc.dram_tensor("w_hh1b", [128, G4], F16, kind="ExternalInput").ap()
    d_fca = nc.dram_tensor("w_fca", [128, OUTPUT_SIZE], F16, kind="ExternalInput").ap()
    d_fcb = nc.dram_tensor("w_fcb", [128, OUTPUT_SIZE], F16, kind="ExternalInput").ap()
    d_b1 = nc.dram_tensor("b1row", [1, G4], F16, kind="ExternalInput").ap()
    d_bfc = nc.dram_tensor("bfcrow", [1, OUTPUT_SIZE], F16, kind="ExternalInput").ap()
    d_h0t = nc.dram_tensor("h0t_init", [128, HIDDEN], F16, kind="ExternalInput").ap()
    d_h1t = nc.dram_tensor("h1t_init", [128, HIDDEN], F16, kind="ExternalInput").ap()
    d_ct0 = nc.dram_tensor("ct0_init", [BLOC, HIDDEN], F32, kind="ExternalInput").ap()
    d_ct1 = nc.dram_tensor("ct1_init", [BLOC, HIDDEN], F32, kind="ExternalInput").ap()
    d_xt = nc.dram_tensor("xt_init", [INPUT_SIZE + 2, BLOC], F16, kind="ExternalInput").ap()

    d_outs = nc.dram_tensor(
        "outs", [BLOC, 16 if timing_ring else steps, INPUT_SIZE + 2], F16,
        kind="ExternalOutput",
    ).ap()
    d_hf = nc.dram_tensor("hf", [2, BLOC, HIDDEN], F16, kind="ExternalOutput").ap()
    d_cf = nc.dram_tensor("cf", [2, BLOC, HIDDEN], F32, kind="ExternalOutput").ap()

    with ExitStack() as ctx:
        tc = ctx.enter_context(tile.TileContext(nc))
        singles = ctx.enter_context(tc.tile_pool(name="singles", bufs=1))
        work = ctx.enter_context(tc.tile_pool(name="work", bufs=3))
        gpsum = ctx.enter_context(tc.tile_pool(name="gpsum", bufs=1, space="PSUM"))
        ppsum = ctx.enter_context(tc.tile_pool(name="ppsum", bufs=1, space="PSUM"))
        tpsum = ctx.enter_context(tc.tile_pool(name="tpsum", bufs=3, space="PSUM"))

        # --- persistent SBUF ----------------------------------------------
        w_ih0 = singles.tile([INPUT_SIZE + 2, G4], F16)
        w_hh0 = [singles.tile([128, G4], F16, tag=f"whh0{c}", name=f"whh0{c}") for c in range(2)]
        w_ih1 = [singles.tile([128, G4], F16, tag=f"wih1{c}", name=f"wih1{c}") for c in range(2)]
        w_hh1 = [singles.tile([128, G4], F16, tag=f"whh1{c}", name=f"whh1{c}") for c in range(2)]
        w_fc = [singles.tile([128, OUTPUT_SIZE], F16, tag=f"wfc{c}", name=f"wfc{c}") for c in range(2)]
        b1row = singles.tile([1, G4], F16)
        bfcrow = singles.tile([1, OUTPUT_SIZE], F16)
        ident = singles.tile([128, 128], F16)
        ones1 = singles.tile([1, 128], F16)
        xsb_pp = [
            singles.tile([128, INPUT_SIZE + 2], F16, tag=f"xsb{i}", name=f"xsb{i}")
            for i in range(2)
        ]
        xT = singles.tile([INPUT_SIZE + 2, BLOC], F16)
        h0T = singles.tile([128, HIDDEN], F16)
        h1T = singles.tile([128, HIDDEN], F16)
        statep = ctx.enter_context(tc.tile_pool(name="statep", bufs=3))

        nc.sync.dma_start(w_ih0, d_ih0)
        nc.sync.dma_start(w_hh0[0], d_hh0a)
        nc.sync.dma_start(w_hh0[1], d_hh0b)
        nc.sync.dma_start(w_ih1[0], d_ih1a)
        nc.sync.dma_start(w_ih1[1], d_ih1b)
        nc.sync.dma_start(w_hh1[0], d_hh1a)
        nc.sync.dma_start(w_hh1[1], d_hh1b)
        nc.sync.dma_start(w_fc[0], d_fca)
        nc.sync.dma_start(w_fc[1], d_fcb)
        nc.sync.dma_start(b1row, d_b1)
        nc.sync.dma_start(bfcrow, d_bfc)
        nc.sync.dma_start(h0T, d_h0t)
        nc.sync.dma_start(h1T, d_h1t)
        nc.sync.dma_start(xT, d_xt)

        make_identity(nc, ident)
        nc.vector.memset(ones1, 1.0)
        for i in range(2):
            nc.vector.memset(xsb_pp[i][:, 0:66], 0.0)
            nc.vector.memset(xsb_pp[i][:, 65:66], 1.0)
            nc.vector.memset(xsb_pp[i][:, 61:62], 1.0)  # step-0 one-hot (overwritten later)
            nc.vector.memset(xsb_pp[i][:, 63:64], -1.0)

        ct = [singles.tile([BLOC, HIDDEN], F32, tag=f"ct_init{l}", name=f"ct_init{l}") for l in range(2)]
        nc.sync.dma_start(ct[0], d_ct0)
        nc.sync.dma_start(ct[1], d_ct1)

        hT = [h0T, h1T]
        w_hh = [w_hh0, w_hh1]
        Hnew_last = [None, None]

        def emit_hh(g, layer, first):
            """hh (and for t=0 only) matmuls opening each chunk's accum group."""
            for c in range(2):
                cs = _chunk_cols(c)
                nc.tensor.matmul(g[:, cs], hT[layer][:, 0:128], w_hh[layer][0][:, cs], start=True, stop=False)
                nc.tensor.matmul(g[:, cs], hT[layer][:, 128:256], w_hh[layer][1][:, cs], start=False, stop=False)

        def emit_cell(g, layer, t):
            """tanh + cell update for one layer; returns nothing (updates hT/ct)."""
            tall = work.tile([BLOC, G4], F16, tag=f"tall{layer}", name=f"tall{layer}")
            ct_new = work.tile([BLOC, HIDDEN], F32, tag=f"ct{layer}", name=f"ct{layer}")
            Hnew = work.tile([BLOC, HIDDEN], F16, tag=f"H{layer}", name=f"H{layer}")
            hT_new = statep.tile([128, HIDDEN], F16, tag=f"hT{layer}", name=f"hT{layer}")
            Hnew_last[layer] = Hnew
            sig = {}
            # ring-critical tanh covers only [i g f]; the o-gate tanh is an
            # ACT-queue filler (H needs it much later), keeping the queue
            # clear for tanh(c)
            for c in range(2):
                o0 = c * 512
                nc.scalar.activation(tall[:, o0 : o0 + 384], g[:, o0 : o0 + 384], AF.Tanh)
            for c in range(2):
                o0 = c * 512
                nc.scalar.activation(tall[:, o0 + 384 : o0 + 512], g[:, o0 + 384 : o0 + 512], AF.Tanh)
            for c in range(2):
                o0 = c * 512
                si = tall[:, o0 : o0 + 128]
                tg = tall[:, o0 + 128 : o0 + 256]
                sf = tall[:, o0 + 256 : o0 + 384]
                hs = slice(c * 128, (c + 1) * 128)
                A = work.tile([BLOC, 128], F16, tag=f"A{layer}{c}", name=f"A{layer}{c}")
                B = work.tile([BLOC, 128], F32, tag=f"B{layer}{c}", name=f"B{layer}{c}")
                # state is D = 2c:  A = 2*sig_i*tanh(g);  B = 2*sig_f*D
                # D_new = 2*c_new = 0.5*B + A;  tanh(c_new) = tanh(0.5*D_new)
                nc.vector.scalar_tensor_tensor(A, si, 1.0, tg, OP.add, OP.mult)
                nc.vector.scalar_tensor_tensor(B, sf, 1.0, ct[layer][:, hs], OP.add, OP.mult)
                nc.vector.scalar_tensor_tensor(ct_new[:, hs], B, 0.5, A, OP.mult, OP.add)
            for c in range(2):
                hs = slice(c * 128, (c + 1) * 128)
                tcn = work.tile([BLOC, 128], F16, tag=f"tc{layer}{c}", name=f"tc{layer}{c}")
                nc.scalar.activation(tcn, ct_new[:, hs], AF.Tanh, scale=0.5)
                sig[c] = tcn
            for c in range(2):
                o0 = c * 512
                so = tall[:, o0 + 384 : o0 + 512]
                hs = slice(c * 128, (c + 1) * 128)
                nc.vector.scalar_tensor_tensor(Hnew[:, hs], so, 1.0, sig[c], OP.add, OP.mult)
                tp = tpsum.tile([128, 128], F16, tag="hT", name="tp")
                nc.tensor.transpose(tp, Hnew[:, hs], ident)
                nc.vector.tensor_copy(hT_new[:, hs], tp)
            ct[layer] = ct_new
            hT[layer] = hT_new

        # prologue: open gates0 accumulation for step 0 with the hh matmuls
        g0 = gpsum.tile([BLOC, G4], F32, tag="g0", name="g0")
        emit_hh(g0, 0, True)

        for t in range(steps):
            xsb = xsb_pp[(t + 1) % 2]  # tile being written this step (for step t+1)

            # close gates0: input matmuls (x'' includes tanh/lse/bias columns)
            for c in range(2):
                cs = _chunk_cols(c)
                nc.tensor.matmul(g0[:, cs], xT, w_ih0[:, cs], start=False, stop=True)

            # open gates1 with its hh + bias matmuls (ready at step start)
            g1 = gpsum.tile([BLOC, G4], F32, tag="g1", name="g1")
            emit_hh(g1, 1, False)
            for c in range(2):
                cs = _chunk_cols(c)
                nc.tensor.matmul(g1[:, cs], ones1, b1row[:, cs], start=False, stop=False)

            # layer-0 cell
            emit_cell(g0, 0, t)

            # close gates1 (k-tile-a work first so nothing head-of-line blocks)
            if t != steps - 1:
                g0n = gpsum.tile([BLOC, G4], F32, tag="g0", name="g0")
            else:
                g0n = None
            for c in range(2):
                cs = _chunk_cols(c)
                nc.tensor.matmul(g1[:, cs], hT[0][:, 0:128], w_ih1[0][:, cs], start=False, stop=False)
            if g0n is not None:
                for c in range(2):
                    cs = _chunk_cols(c)
                    nc.tensor.matmul(g0n[:, cs], hT[0][:, 0:128], w_hh0[0][:, cs], start=True, stop=False)
            for c in range(2):
                cs = _chunk_cols(c)
                nc.tensor.matmul(g1[:, cs], hT[0][:, 128:256], w_ih1[1][:, cs], start=False, stop=True)
            if g0n is not None:
                for c in range(2):
                    cs = _chunk_cols(c)
                    nc.tensor.matmul(g0n[:, cs], hT[0][:, 128:256], w_hh0[1][:, cs], start=False, stop=False)

            # layer-1 cell
            emit_cell(g1, 1, t)

            # FC
            pp = ppsum.tile([BLOC, OUTPUT_SIZE], F32, tag="pp", name="pp")
            nc.tensor.matmul(pp, hT[1][:, 0:128], w_fc[0], start=True, stop=False)
            nc.tensor.matmul(pp, hT[1][:, 128:256], w_fc[1], start=False, stop=False)
            nc.tensor.matmul(pp, ones1, bfcrow, start=False, stop=True)

            # softmax pieces: s = sum(exp(p)), tp-col, lse via bit-hack+Newton
            e63 = work.tile([BLOC, 63], F32, tag="e63", name="e63")
            ssum = work.tile([BLOC, 1], F32, tag="ssum", name="ssum")
            nc.scalar.activation(e63, pp[:, 0:63], AF.Exp, accum_out=ssum)
            nc.scalar.activation(xsb[:, 63:64], pp[:, 63:64], AF.Tanh, scale=0.5)
            conv = work.tile([BLOC, 1], F32, tag="conv", name="conv")
            ny0 = work.tile([BLOC, 1], F32, tag="ny0", name="ny0")
            en = work.tile([BLOC, 1], F32, tag="en", name="en")
            u = work.tile([BLOC, 1], F32, tag="u", name="u")
            nlse = work.tile([BLOC, 1], F32, tag="nlse", name="nlse")
            nc.vector.tensor_copy(conv, ssum.bitcast(I32))
            nc.vector.tensor_scalar(ny0, conv, -LN_A, -LN_B, OP.mult, OP.add)
            nc.scalar.activation(en, ny0, AF.Exp)
            nc.vector.tensor_copy(xsb[:, 0:63], pp[:, 0:63])
            nc.vector.tensor_tensor(u, ssum, en, OP.mult)
            nc.vector.scalar_tensor_tensor(nlse, ny0, 1.0, u, OP.add, OP.subtract)

            # next-step input columns + transpose (ring-critical)
            nc.vector.tensor_copy(xsb[:, 64:65], nlse)
            if t != steps - 1:
                xT_new = statep.tile([INPUT_SIZE + 2, BLOC], F16, tag="xT", name="xT")
                tpx = tpsum.tile([INPUT_SIZE + 2, BLOC], F16, tag="hT", name="tpx")
                nc.tensor.transpose(tpx, xsb, ident)
                nc.vector.tensor_copy(xT_new, tpx)
                xT = xT_new

            # raw per-step output snapshot: host derives log_softmax + dur
            nc.sync.dma_start(
                d_outs[:, (t % 16 if timing_ring else t), :], xsb[:, :]
            )
            g0 = g0n

        # ---------------- final states -----------------------------------
        nc.sync.dma_start(d_hf[0], Hnew_last[0])
        nc.sync.dma_start(d_hf[1], Hnew_last[1])
        nc.sync.dma_start(d_cf[0], ct[0])
        nc.sync.dma_start(d_cf[1], ct[1])

    return nc


def _legalize_waits(nc):
    """walrus accepts at most one sync-wait per compute/DMA instruction.
    Hoist excess waits onto same-engine NoOps inserted just before."""
    uid = 0
    for bb in nc.m.functions[0].blocks:
        il = bb.instructions
        if not any(
            getattr(i, "sync_info", None)
            and i.sync_info.on_wait
            and len(i.sync_info.on_wait) > 1
            for i in il
        ):
            continue
        new = []
        for inst in il:
            si = getattr(inst, "sync_info", None)
            if si and si.on_wait and len(si.on_wait) > 1:
                waits = list(si.on_wait)
                for w in waits[:-1]:
                    nop = mybir.InstEventSemaphore(name=f"waitnop-{uid}", ins=[], outs=[])
                    uid += 1
                    nop.engine = inst.engine
                    nop.sync_info = mybir.SyncInfo(on_wait=[w], on_update=[])
                    new.append(nop)
                inst.sync_info = mybir.SyncInfo(
                    on_wait=[waits[-1]], on_update=si.on_update
                )
            new.append(inst)
        bb.instructions = new


def prep_inputs(inputs, steps=MAX_LENGTH):
    """Host-side prep: returns per-core in_maps for run_bass_kernel_spmd."""
    f = np.asarray
    Wih0 = f(inputs["Wih0"], np.float32)
    Whh0 = f(inputs["Whh0"], np.float32)
    Wih1 = f(inputs["Wih1"], np.float32)
    Whh1 = f(inputs["Whh1"], np.float32)
    Wfc = f(inputs["Wfc"], np.float32)
    bih0 = f(inputs["bih0"], np.float32)
    bhh0 = f(inputs["bhh0"], np.float32)
    bih1 = f(inputs["bih1"], np.float32)
    bhh1 = f(inputs["bhh1"], np.float32)
    bfc = f(inputs["bfc"], np.float32)
    h0 = f(inputs["h0"], np.float32)
    c0 = f(inputs["c0"], np.float32)

    gsp = _GS_PERM[:, None]
    V0 = (Wih0 * _GS[:, None])[_PERM]  # [1024, 64] permuted+scaled
    b0 = ((bih0 + bhh0) * _GS)[_PERM]

    # x'' columns: [p(63) | tanh(p63/2) | -lse | 1]
    w_ih0 = np.zeros((INPUT_SIZE + 2, G4), np.float16)
    w_ih0[0:63] = V0[:, 0:63].T
    w_ih0[63] = 0.5 * V0[:, 63]
    w_ih0[64] = V0[:, 0:63].sum(1)
    w_ih0[65] = b0 + 0.5 * V0[:, 63]

    whh0 = (Whh0[_PERM] * gsp * 0.5).T.astype(np.float16)  # [256, 1024]
    wih1 = (Wih1[_PERM] * gsp * 0.5).T.astype(np.float16)
    whh1 = (Whh1[_PERM] * gsp * 0.5).T.astype(np.float16)
    wfc = (Wfc * 0.5).T.astype(np.float16)  # [256, 64]
    b1row = ((bih1 + bhh1) * _GS)[_PERM][None].astype(np.float16)
    bfcrow = bfc[None].astype(np.float16)

    xt_init = np.zeros((INPUT_SIZE + 2, BLOC), np.float16)
    xt_init[61] = 1.0   # SOS one-hot
    xt_init[63] = -1.0  # tanh col: 0.5*(-1)+0.5 = 0 = x0[63]
    xt_init[65] = 1.0   # const-1 bias row

    shared = dict(
        w_ih0=w_ih0,
        w_hh0a=whh0[0:128].copy(), w_hh0b=whh0[128:256].copy(),
        w_ih1a=wih1[0:128].copy(), w_ih1b=wih1[128:256].copy(),
        w_hh1a=whh1[0:128].copy(), w_hh1b=whh1[128:256].copy(),
        w_fca=wfc[0:128].copy(), w_fcb=wfc[128:256].copy(),
        b1row=b1row, bfcrow=bfcrow,
        xt_init=xt_init,
    )

    in_maps = []
    for i in range(NCORES):
        bsl = slice(i * BLOC, (i + 1) * BLOC)
        H0 = 2.0 * h0[0][bsl]  # [128, 256]
        H1 = 2.0 * h0[1][bsl]
        m = dict(shared)
        m["h0t_init"] = np.concatenate([H0[:, 0:128].T, H0[:, 128:256].T], 1).astype(np.float16)
        m["h1t_init"] = np.concatenate([H1[:, 0:128].T, H1[:, 128:256].T], 1).astype(np.float16)
        m["ct0_init"] = (2.0 * c0[0][bsl]).astype(np.float32)
        m["ct1_init"] = (2.0 * c0[1][bsl]).astype(np.float32)
        in_maps.append(m)
    return in_maps


_BUILD_CACHE = {}


def get_built(steps=MAX_LENGTH):
    if steps not in _BUILD_CACHE:
        _BUILD_CACHE[steps] = build_bass(steps)
    return _BUILD_CACHE[steps]


def run(inputs, steps=MAX_LENGTH, trace=False, **kw):
    nc = get_built(steps)
    if not getattr(nc, "_waits_legalized", False):
        _legalize_waits(nc)
        nc._waits_legalized = True
    in_maps = prep_inputs(inputs, steps)
    res = run_bass_kernel_spmd(nc, in_maps, core_ids=list(range(NCORES)), trace=trace, **kw)
    raw = np.concatenate([r["outs"] for r in res.results], 0).astype(np.float32)
    # raw[:, t] = [p(63) | tanh(p63/2) | -lse | 1]
    outputs = np.empty((BATCH, MAX_LENGTH, OUTPUT_SIZE), np.float32)
    outputs[:, :, 0:63] = raw[:, :, 0:63] + raw[:, :, 64:65]
    outputs[:, :, 63] = 0.5 * raw[:, :, 63] + 0.5
    h_f = np.stack(
        [
            np.concatenate([r["hf"][0].astype(np.float32) for r in res.results], 0),
            np.concatenate([r["hf"][1].astype(np.float32) for r in res.results], 0),
        ]
    ) * 0.5
    c_f = np.stack(
        [
            np.concatenate([r["cf"][0] for r in res.results], 0),
            np.concatenate([r["cf"][1] for r in res.results], 0),
        ]
    ) * 0.5
    return (outputs, h_f, c_f), res


def kernel(**inputs):
    (outputs, h_f, c_f), _ = run(inputs)
    return outputs, h_f, c_f


# revision 31
# speedup vs baseline: 1.0295x; 1.0098x over previous
"""Trainium2 Bass kernel for a 2-layer LSTM decoder (autoregressive, 512 steps).

Strategy (data-parallel over batch):
  - BATCH=1024 split as 128 rows per core across 8 cores; the 128 batch rows
    sit on the SBUF/PSUM partition dimension.
  - Matmuls are state-stationary fp16: out[batch, gates] = lhsT.T @ rhs with
    lhsT = transposed state tiles [K=h/x dims, M=128 batch] and rhs = host-
    pre-transposed weights [K, 4H] streamed N=512 per PSUM bank.
  - Single ACT table set (exp_and_others): sigmoid(x) computed as
    0.5*tanh(x/2)+0.5 with the 0.5 gate-scales folded into the weights and the
    (s+1) affines folded into fused scalar_tensor_tensor ops; h-state kept
    doubled (H=2h) with h-side weights pre-halved so no extra scaling ops.
  - log-sum-exp for log_softmax: exp with ACT accum_out gives s = sum(exp(p));
    ln(s) via exponent bit-hack initial guess + one Newton step (uses exp,
    stays in the same ACT table set).
  - The -lse and sigmoid(p63) contributions to the next step's input are
    applied as a K=2 rank-1 matmul so they stay off the critical path.
"""

import math
import sys

import numpy as np

if "/opt/trn_rl_repo" not in sys.path:
    sys.path.insert(0, "/opt/trn_rl_repo")

import concourse.bass as bass
import concourse.mybir as mybir
import concourse.tile as tile
from concourse.bass_utils import run_bass_kernel_spmd
from concourse.masks import make_identity

F16 = mybir.dt.float16
F32 = mybir.dt.float32
I32 = mybir.dt.int32
AF = mybir.ActivationFunctionType
OP = mybir.AluOpType

INPUT_SIZE = 64
HIDDEN = 256
OUTPUT_SIZE = 64
MAX_LENGTH = 512
BATCH = 1024
NCORES = 8
BLOC = BATCH // NCORES  # 128
G4 = 4 * HIDDEN  # 1024

# ln(x) ~= A*float(bitcast_i32(x)) + B, then one Newton step y += x*exp(-y)-1
LN2 = 0.6931471805599453
LN_A = LN2 / (1 << 23)
LN_B = LN2 * (0.0430357 - 127.0)

# gate permutation: torch order rows [i f g o] each 256 -> chunked layout
# [i_a f_a g_a o_a | i_b f_b g_b o_b] with a = h-dims 0:128, b = 128:256
_PERM = np.concatenate(
    [
        np.r_[0:128], np.r_[512:640], np.r_[256:384], np.r_[768:896],
        np.r_[128:256], np.r_[640:768], np.r_[384:512], np.r_[896:1024],
    ]
)
# gate scale: 0.5 for i,f,o (sigmoid-via-tanh), 1.0 for g (plain tanh)
_GS = np.ones(G4, np.float32)
_GS[0:256] = 0.5
_GS[256:512] = 0.5
_GS[768:1024] = 0.5
_GS_PERM = _GS[_PERM]


def _chunk_cols(c):
    return slice(c * 512, (c + 1) * 512)


def build_bass(steps=MAX_LENGTH, timing_ring=False):
    """Build the single-core Bass program (identical across cores).

    timing_ring=True shrinks the outputs tensor to a 16-step ring so timing
    runs avoid the large host<->device transfer (outputs then invalid).
    """
    from contextlib import ExitStack

    nc = bass.Bass("TRN2", target_bir_lowering=False, debug=False)

    # --- DRAM I/O ----------------------------------------------------------
    d_ih0 = nc.dram_tensor("w_ih0", [INPUT_SIZE + 2, G4], F16, kind="ExternalInput").ap()
    d_hh0a = nc.dram_tensor("w_hh0a", [128, G4], F16, kind="ExternalInput").ap()
    d_hh0b = nc.dram_tensor("w_hh0b", [128, G4], F16, kind="ExternalInput").ap()
    d_ih1a = nc.dram_tensor("w_ih1a", [128, G4], F16, kind="ExternalInput").ap()
    d_ih1b = nc.dram_tensor("w_ih1b", [128, G4], F16, kind="ExternalInput").ap()
    d_hh1a = nc.dram_tensor("w_hh1a", [128, G4], F16, kind="ExternalInput").ap()
    d_hh1b = nc.dram_tensor("w_hh1b", [128, G4], F16, kind="ExternalInput").ap()
    d_fca = nc.dram_tensor("w_fca", [128, OUTPUT_SIZE], F16, kind="ExternalInput").ap()
    d_fcb = nc.dram_tensor("w_fcb", [128, OUTPUT_SIZE], F16, kind="ExternalInput").ap()
    d_b1 = nc.dram_tensor("b1row", [1, G4], F16, kind="ExternalInput").ap()
    d_bfc = nc.dram_tensor("bfcrow", [1, OUTPUT_SIZE], F16, kind="ExternalInput").ap()
    d_h0t = nc.dram_tensor("h0t_init", [128, HIDDEN], F16, kind="ExternalInput").ap()
    d_h1t = nc.dram_tensor("h1t_init", [128, HIDDEN], F16, kind="ExternalInput").ap()
    d_ct0 = nc.dram_tensor("ct0_init", [BLOC, HIDDEN], F32, kind="ExternalInput").ap()
    d_ct1 = nc.dram_tensor("ct1_init", [BLOC, HIDDEN], F32, kind="ExternalInput").ap()
    d_xt = nc.dram_tensor("xt_init", [INPUT_SIZE + 2, BLOC], F16, kind="ExternalInput").ap()

    d_outs = nc.dram_tensor(
        "outs", [BLOC, 16 if timing_ring else steps, INPUT_SIZE + 2], F16,
        kind="ExternalOutput",
    ).ap()
    d_hf = nc.dram_tensor("hf", [2, BLOC, HIDDEN], F16, kind="ExternalOutput").ap()
    d_cf = nc.dram_tensor("cf", [2, BLOC, HIDDEN], F32, kind="ExternalOutput").ap()

    with ExitStack() as ctx:
        tc = ctx.enter_context(tile.TileContext(nc))
        singles = ctx.enter_context(tc.tile_pool(name="singles", bufs=1))
        work = ctx.enter_context(tc.tile_pool(name="work", bufs=3))
        gpsum = ctx.enter_context(tc.tile_pool(name="gpsum", bufs=1, space="PSUM"))
        ppsum = ctx.enter_context(tc.tile_pool(name="ppsum", bufs=1, space="PSUM"))
        tpsum = ctx.enter_context(tc.tile_pool(name="tpsum", bufs=3, space="PSUM"))

        # --- persistent SBUF ----------------------------------------------
        w_ih0 = singles.tile([INPUT_SIZE + 2, G4], F16)
        w_hh0 = [singles.tile([128, G4], F16, tag=f"whh0{c}", name=f"whh0{c}") for c in range(2)]
        w_ih1 = [singles.tile([128, G4], F16, tag=f"wih1{c}", name=f"wih1{c}") for c in range(2)]
        w_hh1 = [singles.tile([128, G4], F16, tag=f"whh1{c}", name=f"whh1{c}") for c in range(2)]
        w_fc = [singles.tile([128, OUTPUT_SIZE], F16, tag=f"wfc{c}", name=f"wfc{c}") for c in range(2)]
        b1row = singles.tile([1, G4], F16)
        bfcrow = singles.tile([1, OUTPUT_SIZE], F16)
        ident = singles.tile([128, 128], F16)
        ones1 = singles.tile([1, 128], F16)
        xsb_pp = [
            singles.tile([128, INPUT_SIZE + 2], F16, tag=f"xsb{i}", name=f"xsb{i}")
            for i in range(2)
        ]
        xT = singles.tile([INPUT_SIZE + 2, BLOC], F16)
        h0T = singles.tile([128, HIDDEN], F16)
        h1T = singles.tile([128, HIDDEN], F16)
        statep = ctx.enter_context(tc.tile_pool(name="statep", bufs=3))

        nc.sync.dma_start(w_ih0, d_ih0)
        nc.sync.dma_start(w_hh0[0], d_hh0a)
        nc.sync.dma_start(w_hh0[1], d_hh0b)
        nc.sync.dma_start(w_ih1[0], d_ih1a)
        nc.sync.dma_start(w_ih1[1], d_ih1b)
        nc.sync.dma_start(w_hh1[0], d_hh1a)
        nc.sync.dma_start(w_hh1[1], d_hh1b)
        nc.sync.dma_start(w_fc[0], d_fca)
        nc.sync.dma_start(w_fc[1], d_fcb)
        nc.sync.dma_start(b1row, d_b1)
        nc.sync.dma_start(bfcrow, d_bfc)
        nc.sync.dma_start(h0T, d_h0t)
        nc.sync.dma_start(h1T, d_h1t)
        nc.sync.dma_start(xT, d_xt)

        make_identity(nc, ident)
        nc.vector.memset(ones1, 1.0)
        for i in range(2):
            nc.vector.memset(xsb_pp[i][:, 0:66], 0.0)
            nc.vector.memset(xsb_pp[i][:, 65:66], 1.0)
            nc.vector.memset(xsb_pp[i][:, 61:62], 1.0)  # step-0 one-hot (overwritten later)
            nc.vector.memset(xsb_pp[i][:, 63:64], -1.0)

        ct = [singles.tile([BLOC, HIDDEN], F32, tag=f"ct_init{l}", name=f"ct_init{l}") for l in range(2)]
        nc.sync.dma_start(ct[0], d_ct0)
        nc.sync.dma_start(ct[1], d_ct1)

        hT = [h0T, h1T]
        w_hh = [w_hh0, w_hh1]
        Hnew_last = [None, None]

        def emit_hh(g, layer, first):
            """hh (and for t=0 only) matmuls opening each chunk's accum group."""
            for c in range(2):
                cs = _chunk_cols(c)
                nc.tensor.matmul(g[:, cs], hT[layer][:, 0:128], w_hh[layer][0][:, cs], start=True, stop=False)
                nc.tensor.matmul(g[:, cs], hT[layer][:, 128:256], w_hh[layer][1][:, cs], start=False, stop=False)

        def emit_cell(g, layer, t):
            """tanh + cell update for one layer; returns nothing (updates hT/ct)."""
            tall = work.tile([BLOC, G4], F16, tag=f"tall{layer}", name=f"tall{layer}")
            ct_new = work.tile([BLOC, HIDDEN], F32, tag=f"ct{layer}", name=f"ct{layer}")
            Hnew = work.tile([BLOC, HIDDEN], F16, tag=f"H{layer}", name=f"H{layer}")
            hT_new = statep.tile([128, HIDDEN], F16, tag=f"hT{layer}", name=f"hT{layer}")
            Hnew_last[layer] = Hnew
            sig = {}
            # ring-critical tanh covers only [i g f]; the o-gate tanh is an
            # ACT-queue filler (H needs it much later), keeping the queue
            # clear for tanh(c)
            for c in range(2):
                o0 = c * 512
                nc.scalar.activation(tall[:, o0 : o0 + 384], g[:, o0 : o0 + 384], AF.Tanh)
            for c in range(2):
                o0 = c * 512
                nc.scalar.activation(tall[:, o0 + 384 : o0 + 512], g[:, o0 + 384 : o0 + 512], AF.Tanh)
            for c in range(2):
                o0 = c * 512
                si = tall[:, o0 : o0 + 128]
                tg = tall[:, o0 + 128 : o0 + 256]
                sf = tall[:, o0 + 256 : o0 + 384]
                hs = slice(c * 128, (c + 1) * 128)
                A = work.tile([BLOC, 128], F16, tag=f"A{layer}{c}", name=f"A{layer}{c}")
                B = work.tile([BLOC, 128], F32, tag=f"B{layer}{c}", name=f"B{layer}{c}")
                # state is D = 2c:  A = 2*sig_i*tanh(g);  B = 2*sig_f*D
                # D_new = 2*c_new = 0.5*B + A;  tanh(c_new) = tanh(0.5*D_new)
                nc.vector.scalar_tensor_tensor(A, si, 1.0, tg, OP.add, OP.mult)
                nc.vector.scalar_tensor_tensor(B, sf, 1.0, ct[layer][:, hs], OP.add, OP.mult)
                nc.vector.scalar_tensor_tensor(ct_new[:, hs], B, 0.5, A, OP.mult, OP.add)
            for c in range(2):
                hs = slice(c * 128, (c + 1) * 128)
                tcn = work.tile([BLOC, 128], F16, tag=f"tc{layer}{c}", name=f"tc{layer}{c}")
                nc.scalar.activation(tcn, ct_new[:, hs], AF.Tanh, scale=0.5)
                sig[c] = tcn
            for c in range(2):
                o0 = c * 512
                so = tall[:, o0 + 384 : o0 + 512]
                hs = slice(c * 128, (c + 1) * 128)
                nc.vector.scalar_tensor_tensor(Hnew[:, hs], so, 1.0, sig[c], OP.add, OP.mult)
                tp = tpsum.tile([128, 128], F16, tag="hT", name="tp")
                nc.tensor.transpose(tp, Hnew[:, hs], ident)
                nc.vector.tensor_copy(hT_new[:, hs], tp)
            ct[layer] = ct_new
            hT[layer] = hT_new

        # prologue: open gates0 accumulation for step 0 with the hh matmuls
        g0 = gpsum.tile([BLOC, G4], F32, tag="g0", name="g0")
        emit_hh(g0, 0, True)

        for t in range(steps):
            xsb = xsb_pp[(t + 1) % 2]  # tile being written this step (for step t+1)

            # close gates0: input matmuls (x'' includes tanh/lse/bias columns)
            for c in range(2):
                cs = _chunk_cols(c)
                nc.tensor.matmul(g0[:, cs], xT, w_ih0[:, cs], start=False, stop=True)

            # open gates1 with its hh + bias matmuls (ready at step start)
            g1 = gpsum.tile([BLOC, G4], F32, tag="g1", name="g1")
            emit_hh(g1, 1, False)
            for c in range(2):
                cs = _chunk_cols(c)
                nc.tensor.matmul(g1[:, cs], ones1, b1row[:, cs], start=False, stop=False)

            # layer-0 cell
            emit_cell(g0, 0, t)

            # close gates1 k-tile-major; next-step g0 hh fillers strictly AFTER
            # the ring-critical ih matmuls so they never win scheduler ties
            if t != steps - 1:
                g0n = gpsum.tile([BLOC, G4], F32, tag="g0", name="g0")
            else:
                g0n = None
            for c in range(2):
                cs = _chunk_cols(c)
                nc.tensor.matmul(g1[:, cs], hT[0][:, 0:128], w_ih1[0][:, cs], start=False, stop=False)
            for c in range(2):
                cs = _chunk_cols(c)
                nc.tensor.matmul(g1[:, cs], hT[0][:, 128:256], w_ih1[1][:, cs], start=False, stop=True)
            if g0n is not None:
                for c in range(2):
                    cs = _chunk_cols(c)
                    nc.tensor.matmul(g0n[:, cs], hT[0][:, 0:128], w_hh0[0][:, cs], start=True, stop=False)
                for c in range(2):
                    cs = _chunk_cols(c)
                    nc.tensor.matmul(g0n[:, cs], hT[0][:, 128:256], w_hh0[1][:, cs], start=False, stop=False)

            # layer-1 cell
            emit_cell(g1, 1, t)

            # FC
            pp = ppsum.tile([BLOC, OUTPUT_SIZE], F32, tag="pp", name="pp")
            nc.tensor.matmul(pp, hT[1][:, 0:128], w_fc[0], start=True, stop=False)
            nc.tensor.matmul(pp, hT[1][:, 128:256], w_fc[1], start=False, stop=False)
            nc.tensor.matmul(pp, ones1, bfcrow, start=False, stop=True)

            # softmax pieces: s = sum(exp(p)), tp-col, lse via bit-hack+Newton
            e63 = work.tile([BLOC, 63], F32, tag="e63", name="e63")
            ssum = work.tile([BLOC, 1], F32, tag="ssum", name="ssum")
            nc.scalar.activation(e63, pp[:, 0:63], AF.Exp, accum_out=ssum)
            nc.scalar.activation(xsb[:, 63:64], pp[:, 63:64], AF.Tanh, scale=0.5)
            conv = work.tile([BLOC, 1], F32, tag="conv", name="conv")
            ny0 = work.tile([BLOC, 1], F32, tag="ny0", name="ny0")
            en = work.tile([BLOC, 1], F32, tag="en", name="en")
            u = work.tile([BLOC, 1], F32, tag="u", name="u")
            nlse = work.tile([BLOC, 1], F32, tag="nlse", name="nlse")
            nc.vector.tensor_copy(conv, ssum.bitcast(I32))
            nc.vector.tensor_scalar(ny0, conv, -LN_A, -LN_B, OP.mult, OP.add)
            nc.scalar.activation(en, ny0, AF.Exp)
            nc.vector.tensor_copy(xsb[:, 0:63], pp[:, 0:63])
            nc.vector.tensor_tensor(u, ssum, en, OP.mult)
            nc.vector.scalar_tensor_tensor(nlse, ny0, 1.0, u, OP.add, OP.subtract)

            # next-step input columns + transpose (ring-critical)
            nc.vector.tensor_copy(xsb[:, 64:65], nlse)
            if t != steps - 1:
                xT_new = statep.tile([INPUT_SIZE + 2, BLOC], F16, tag="xT", name="xT")
                tpx = tpsum.tile([INPUT_SIZE + 2, BLOC], F16, tag="hT", name="tpx")
                nc.tensor.transpose(tpx, xsb, ident)
                nc.vector.tensor_copy(xT_new, tpx)
                xT = xT_new

            # raw per-step output snapshot: host derives log_softmax + dur
            nc.sync.dma_start(
                d_outs[:, (t % 16 if timing_ring else t), :], xsb[:, :]
            )
            g0 = g0n

        # ---------------- final states -----------------------------------
        nc.sync.dma_start(d_hf[0], Hnew_last[0])
        nc.sync.dma_start(d_hf[1], Hnew_last[1])
        nc.sync.dma_start(d_cf[0], ct[0])
        nc.sync.dma_start(d_cf[1], ct[1])

    return nc


def _legalize_waits(nc):
    """walrus accepts at most one sync-wait per compute/DMA instruction.
    Hoist excess waits onto same-engine NoOps inserted just before."""
    uid = 0
    for bb in nc.m.functions[0].blocks:
        il = bb.instructions
        if not any(
            getattr(i, "sync_info", None)
            and i.sync_info.on_wait
            and len(i.sync_info.on_wait) > 1
            for i in il
        ):
            continue
        new = []
        for inst in il:
            si = getattr(inst, "sync_info", None)
            if si and si.on_wait and len(si.on_wait) > 1:
                waits = list(si.on_wait)
                for w in waits[:-1]:
                    nop = mybir.InstEventSemaphore(name=f"waitnop-{uid}", ins=[], outs=[])
                    uid += 1
                    nop.engine = inst.engine
                    nop.sync_info = mybir.SyncInfo(on_wait=[w], on_update=[])
                    new.append(nop)
                inst.sync_info = mybir.SyncInfo(
                    on_wait=[waits[-1]], on_update=si.on_update
                )
            new.append(inst)
        bb.instructions = new


def prep_inputs(inputs, steps=MAX_LENGTH):
    """Host-side prep: returns per-core in_maps for run_bass_kernel_spmd."""
    f = np.asarray
    Wih0 = f(inputs["Wih0"], np.float32)
    Whh0 = f(inputs["Whh0"], np.float32)
    Wih1 = f(inputs["Wih1"], np.float32)
    Whh1 = f(inputs["Whh1"], np.float32)
    Wfc = f(inputs["Wfc"], np.float32)
    bih0 = f(inputs["bih0"], np.float32)
    bhh0 = f(inputs["bhh0"], np.float32)
    bih1 = f(inputs["bih1"], np.float32)
    bhh1 = f(inputs["bhh1"], np.float32)
    bfc = f(inputs["bfc"], np.float32)
    h0 = f(inputs["h0"], np.float32)
    c0 = f(inputs["c0"], np.float32)

    gsp = _GS_PERM[:, None]
    V0 = (Wih0 * _GS[:, None])[_PERM]  # [1024, 64] permuted+scaled
    b0 = ((bih0 + bhh0) * _GS)[_PERM]

    # x'' columns: [p(63) | tanh(p63/2) | -lse | 1]
    w_ih0 = np.zeros((INPUT_SIZE + 2, G4), np.float16)
    w_ih0[0:63] = V0[:, 0:63].T
    w_ih0[63] = 0.5 * V0[:, 63]
    w_ih0[64] = V0[:, 0:63].sum(1)
    w_ih0[65] = b0 + 0.5 * V0[:, 63]

    whh0 = (Whh0[_PERM] * gsp * 0.5).T.astype(np.float16)  # [256, 1024]
    wih1 = (Wih1[_PERM] * gsp * 0.5).T.astype(np.float16)
    whh1 = (Whh1[_PERM] * gsp * 0.5).T.astype(np.float16)
    wfc = (Wfc * 0.5).T.astype(np.float16)  # [256, 64]
    b1row = ((bih1 + bhh1) * _GS)[_PERM][None].astype(np.float16)
    bfcrow = bfc[None].astype(np.float16)

    xt_init = np.zeros((INPUT_SIZE + 2, BLOC), np.float16)
    xt_init[61] = 1.0   # SOS one-hot
    xt_init[63] = -1.0  # tanh col: 0.5*(-1)+0.5 = 0 = x0[63]
    xt_init[65] = 1.0   # const-1 bias row

    shared = dict(
        w_ih0=w_ih0,
        w_hh0a=whh0[0:128].copy(), w_hh0b=whh0[128:256].copy(),
        w_ih1a=wih1[0:128].copy(), w_ih1b=wih1[128:256].copy(),
        w_hh1a=whh1[0:128].copy(), w_hh1b=whh1[128:256].copy(),
        w_fca=wfc[0:128].copy(), w_fcb=wfc[128:256].copy(),
        b1row=b1row, bfcrow=bfcrow,
        xt_init=xt_init,
    )

    in_maps = []
    for i in range(NCORES):
        bsl = slice(i * BLOC, (i + 1) * BLOC)
        H0 = 2.0 * h0[0][bsl]  # [128, 256]
        H1 = 2.0 * h0[1][bsl]
        m = dict(shared)
        m["h0t_init"] = np.concatenate([H0[:, 0:128].T, H0[:, 128:256].T], 1).astype(np.float16)
        m["h1t_init"] = np.concatenate([H1[:, 0:128].T, H1[:, 128:256].T], 1).astype(np.float16)
        m["ct0_init"] = (2.0 * c0[0][bsl]).astype(np.float32)
        m["ct1_init"] = (2.0 * c0[1][bsl]).astype(np.float32)
        in_maps.append(m)
    return in_maps


_BUILD_CACHE = {}


def get_built(steps=MAX_LENGTH):
    if steps not in _BUILD_CACHE:
        _BUILD_CACHE[steps] = build_bass(steps)
    return _BUILD_CACHE[steps]


def run(inputs, steps=MAX_LENGTH, trace=False, **kw):
    nc = get_built(steps)
    if not getattr(nc, "_waits_legalized", False):
        _legalize_waits(nc)
        nc._waits_legalized = True
    in_maps = prep_inputs(inputs, steps)
    res = run_bass_kernel_spmd(nc, in_maps, core_ids=list(range(NCORES)), trace=trace, **kw)
    raw = np.concatenate([r["outs"] for r in res.results], 0).astype(np.float32)
    # raw[:, t] = [p(63) | tanh(p63/2) | -lse | 1]
    outputs = np.empty((BATCH, MAX_LENGTH, OUTPUT_SIZE), np.float32)
    outputs[:, :, 0:63] = raw[:, :, 0:63] + raw[:, :, 64:65]
    outputs[:, :, 63] = 0.5 * raw[:, :, 63] + 0.5
    h_f = np.stack(
        [
            np.concatenate([r["hf"][0].astype(np.float32) for r in res.results], 0),
            np.concatenate([r["hf"][1].astype(np.float32) for r in res.results], 0),
        ]
    ) * 0.5
    c_f = np.stack(
        [
            np.concatenate([r["cf"][0] for r in res.results], 0),
            np.concatenate([r["cf"][1] for r in res.results], 0),
        ]
    ) * 0.5
    return (outputs, h_f, c_f), res


def kernel(**inputs):
    (outputs, h_f, c_f), _ = run(inputs)
    return outputs, h_f, c_f


# revision 32
# speedup vs baseline: 1.0323x; 1.0027x over previous
"""Trainium2 Bass kernel for a 2-layer LSTM decoder (autoregressive, 512 steps).

Strategy (data-parallel over batch):
  - BATCH=1024 split as 128 rows per core across 8 cores; the 128 batch rows
    sit on the SBUF/PSUM partition dimension.
  - Matmuls are state-stationary fp16: out[batch, gates] = lhsT.T @ rhs with
    lhsT = transposed state tiles [K=h/x dims, M=128 batch] and rhs = host-
    pre-transposed weights [K, 4H] streamed N=512 per PSUM bank.
  - Single ACT table set (exp_and_others): sigmoid(x) computed as
    0.5*tanh(x/2)+0.5 with the 0.5 gate-scales folded into the weights and the
    (s+1) affines folded into fused scalar_tensor_tensor ops; h-state kept
    doubled (H=2h) with h-side weights pre-halved so no extra scaling ops.
  - log-sum-exp for log_softmax: exp with ACT accum_out gives s = sum(exp(p));
    ln(s) via exponent bit-hack initial guess + one Newton step (uses exp,
    stays in the same ACT table set).
  - The -lse and sigmoid(p63) contributions to the next step's input are
    applied as a K=2 rank-1 matmul so they stay off the critical path.
"""

import math
import sys

import numpy as np

if "/opt/trn_rl_repo" not in sys.path:
    sys.path.insert(0, "/opt/trn_rl_repo")

import concourse.bass as bass
import concourse.mybir as mybir
import concourse.tile as tile
from concourse.bass_utils import run_bass_kernel_spmd
from concourse.masks import make_identity

F16 = mybir.dt.float16
F32 = mybir.dt.float32
I32 = mybir.dt.int32
AF = mybir.ActivationFunctionType
OP = mybir.AluOpType

INPUT_SIZE = 64
HIDDEN = 256
OUTPUT_SIZE = 64
MAX_LENGTH = 512
BATCH = 1024
NCORES = 8
BLOC = BATCH // NCORES  # 128
G4 = 4 * HIDDEN  # 1024

# ln(x) ~= A*float(bitcast_i32(x)) + B, then one Newton step y += x*exp(-y)-1
LN2 = 0.6931471805599453
LN_A = LN2 / (1 << 23)
LN_B = LN2 * (0.0430357 - 127.0)

# gate permutation: torch order rows [i f g o] each 256 -> chunked layout
# [i_a f_a g_a o_a | i_b f_b g_b o_b] with a = h-dims 0:128, b = 128:256
_PERM = np.concatenate(
    [
        np.r_[0:128], np.r_[512:640], np.r_[256:384], np.r_[768:896],
        np.r_[128:256], np.r_[640:768], np.r_[384:512], np.r_[896:1024],
    ]
)
# gate scale: 0.5 for i,f,o (sigmoid-via-tanh), 1.0 for g (plain tanh)
_GS = np.ones(G4, np.float32)
_GS[0:256] = 0.5
_GS[256:512] = 0.5
_GS[768:1024] = 0.5
_GS_PERM = _GS[_PERM]


def _chunk_cols(c):
    return slice(c * 512, (c + 1) * 512)


def build_bass(steps=MAX_LENGTH, timing_ring=False):
    """Build the single-core Bass program (identical across cores).

    timing_ring=True shrinks the outputs tensor to a 16-step ring so timing
    runs avoid the large host<->device transfer (outputs then invalid).
    """
    from contextlib import ExitStack

    nc = bass.Bass("TRN2", target_bir_lowering=False, debug=False)

    # --- DRAM I/O ----------------------------------------------------------
    d_ih0 = nc.dram_tensor("w_ih0", [INPUT_SIZE + 2, G4], F16, kind="ExternalInput").ap()
    d_hh0a = nc.dram_tensor("w_hh0a", [128, G4], F16, kind="ExternalInput").ap()
    d_hh0b = nc.dram_tensor("w_hh0b", [128, G4], F16, kind="ExternalInput").ap()
    d_ih1a = nc.dram_tensor("w_ih1a", [128, G4], F16, kind="ExternalInput").ap()
    d_ih1b = nc.dram_tensor("w_ih1b", [128, G4], F16, kind="ExternalInput").ap()
    d_hh1a = nc.dram_tensor("w_hh1a", [128, G4], F16, kind="ExternalInput").ap()
    d_hh1b = nc.dram_tensor("w_hh1b", [128, G4], F16, kind="ExternalInput").ap()
    d_fca = nc.dram_tensor("w_fca", [128, OUTPUT_SIZE], F16, kind="ExternalInput").ap()
    d_fcb = nc.dram_tensor("w_fcb", [128, OUTPUT_SIZE], F16, kind="ExternalInput").ap()
    d_b1 = nc.dram_tensor("b1row", [1, G4], F16, kind="ExternalInput").ap()
    d_bfc = nc.dram_tensor("bfcrow", [1, OUTPUT_SIZE], F16, kind="ExternalInput").ap()
    d_h0t = nc.dram_tensor("h0t_init", [128, HIDDEN], F16, kind="ExternalInput").ap()
    d_h1t = nc.dram_tensor("h1t_init", [128, HIDDEN], F16, kind="ExternalInput").ap()
    d_ct0 = nc.dram_tensor("ct0_init", [BLOC, HIDDEN], F32, kind="ExternalInput").ap()
    d_ct1 = nc.dram_tensor("ct1_init", [BLOC, HIDDEN], F32, kind="ExternalInput").ap()
    d_xt = nc.dram_tensor("xt_init", [INPUT_SIZE + 2, BLOC], F16, kind="ExternalInput").ap()

    d_outs = nc.dram_tensor(
        "outs", [BLOC, 16 if timing_ring else steps, INPUT_SIZE + 2], F16,
        kind="ExternalOutput",
    ).ap()
    d_hf = nc.dram_tensor("hf", [2, BLOC, HIDDEN], F16, kind="ExternalOutput").ap()
    d_cf = nc.dram_tensor("cf", [2, BLOC, HIDDEN], F32, kind="ExternalOutput").ap()

    with ExitStack() as ctx:
        tc = ctx.enter_context(tile.TileContext(nc))
        singles = ctx.enter_context(tc.tile_pool(name="singles", bufs=1))
        work = ctx.enter_context(tc.tile_pool(name="work", bufs=3))
        gpsum = ctx.enter_context(tc.tile_pool(name="gpsum", bufs=1, space="PSUM"))
        ppsum = ctx.enter_context(tc.tile_pool(name="ppsum", bufs=1, space="PSUM"))
        tpsum = ctx.enter_context(tc.tile_pool(name="tpsum", bufs=3, space="PSUM"))

        # --- persistent SBUF ----------------------------------------------
        w_ih0 = singles.tile([INPUT_SIZE + 2, G4], F16)
        w_hh0 = [singles.tile([128, G4], F16, tag=f"whh0{c}", name=f"whh0{c}") for c in range(2)]
        w_ih1 = [singles.tile([128, G4], F16, tag=f"wih1{c}", name=f"wih1{c}") for c in range(2)]
        w_hh1 = [singles.tile([128, G4], F16, tag=f"whh1{c}", name=f"whh1{c}") for c in range(2)]
        w_fc = [singles.tile([128, OUTPUT_SIZE], F16, tag=f"wfc{c}", name=f"wfc{c}") for c in range(2)]
        b1row = singles.tile([1, G4], F16)
        bfcrow = singles.tile([1, OUTPUT_SIZE], F16)
        ident = singles.tile([128, 128], F16)
        ones1 = singles.tile([1, 128], F16)
        xsb_pp = [
            singles.tile([128, INPUT_SIZE + 2], F16, tag=f"xsb{i}", name=f"xsb{i}")
            for i in range(2)
        ]
        xT = singles.tile([INPUT_SIZE + 2, BLOC], F16)
        h0T = singles.tile([128, HIDDEN], F16)
        h1T = singles.tile([128, HIDDEN], F16)
        statep = ctx.enter_context(tc.tile_pool(name="statep", bufs=3))

        nc.sync.dma_start(w_ih0, d_ih0)
        nc.sync.dma_start(w_hh0[0], d_hh0a)
        nc.sync.dma_start(w_hh0[1], d_hh0b)
        nc.sync.dma_start(w_ih1[0], d_ih1a)
        nc.sync.dma_start(w_ih1[1], d_ih1b)
        nc.sync.dma_start(w_hh1[0], d_hh1a)
        nc.sync.dma_start(w_hh1[1], d_hh1b)
        nc.sync.dma_start(w_fc[0], d_fca)
        nc.sync.dma_start(w_fc[1], d_fcb)
        nc.sync.dma_start(b1row, d_b1)
        nc.sync.dma_start(bfcrow, d_bfc)
        nc.sync.dma_start(h0T, d_h0t)
        nc.sync.dma_start(h1T, d_h1t)
        nc.sync.dma_start(xT, d_xt)

        make_identity(nc, ident)
        nc.vector.memset(ones1, 1.0)
        for i in range(2):
            nc.vector.memset(xsb_pp[i][:, 0:66], 0.0)
            nc.vector.memset(xsb_pp[i][:, 65:66], 1.0)
            nc.vector.memset(xsb_pp[i][:, 61:62], 1.0)  # step-0 one-hot (overwritten later)
            nc.vector.memset(xsb_pp[i][:, 63:64], -1.0)

        ct = [singles.tile([BLOC, HIDDEN], F32, tag=f"ct_init{l}", name=f"ct_init{l}") for l in range(2)]
        nc.sync.dma_start(ct[0], d_ct0)
        nc.sync.dma_start(ct[1], d_ct1)

        hT = [h0T, h1T]
        w_hh = [w_hh0, w_hh1]
        Hnew_last = [None, None]

        def emit_hh(g, layer, first):
            """hh (and for t=0 only) matmuls opening each chunk's accum group."""
            for c in range(2):
                cs = _chunk_cols(c)
                nc.tensor.matmul(g[:, cs], hT[layer][:, 0:128], w_hh[layer][0][:, cs], start=True, stop=False)
                nc.tensor.matmul(g[:, cs], hT[layer][:, 128:256], w_hh[layer][1][:, cs], start=False, stop=False)

        def emit_cell(g, layer, t):
            """tanh + cell update for one layer; returns nothing (updates hT/ct)."""
            tall = work.tile([BLOC, G4], F16, tag=f"tall{layer}", name=f"tall{layer}")
            ct_new = work.tile([BLOC, HIDDEN], F32, tag=f"ct{layer}", name=f"ct{layer}")
            Hnew = work.tile([BLOC, HIDDEN], F16, tag=f"H{layer}", name=f"H{layer}")
            hT_new = statep.tile([128, HIDDEN], F16, tag=f"hT{layer}", name=f"hT{layer}")
            Hnew_last[layer] = Hnew
            sig = {}
            # ring-critical tanh covers only [i g f]; the o-gate tanh is an
            # ACT-queue filler (H needs it much later), keeping the queue
            # clear for tanh(c)
            for c in range(2):
                o0 = c * 512
                nc.scalar.activation(tall[:, o0 : o0 + 384], g[:, o0 : o0 + 384], AF.Tanh)
            for c in range(2):
                o0 = c * 512
                nc.scalar.activation(tall[:, o0 + 384 : o0 + 512], g[:, o0 + 384 : o0 + 512], AF.Tanh)
            for c in range(2):
                o0 = c * 512
                si = tall[:, o0 : o0 + 128]
                tg = tall[:, o0 + 128 : o0 + 256]
                sf = tall[:, o0 + 256 : o0 + 384]
                hs = slice(c * 128, (c + 1) * 128)
                A = work.tile([BLOC, 128], F16, tag=f"A{layer}{c}", name=f"A{layer}{c}")
                B = work.tile([BLOC, 128], F32, tag=f"B{layer}{c}", name=f"B{layer}{c}")
                # state is D = 2c:  A = 2*sig_i*tanh(g);  B = 2*sig_f*D
                # D_new = 2*c_new = 0.5*B + A;  tanh(c_new) = tanh(0.5*D_new)
                nc.vector.scalar_tensor_tensor(A, si, 1.0, tg, OP.add, OP.mult)
                nc.vector.scalar_tensor_tensor(B, sf, 1.0, ct[layer][:, hs], OP.add, OP.mult)
                nc.vector.scalar_tensor_tensor(ct_new[:, hs], B, 0.5, A, OP.mult, OP.add)
            for c in range(2):
                hs = slice(c * 128, (c + 1) * 128)
                tcn = work.tile([BLOC, 128], F16, tag=f"tc{layer}{c}", name=f"tc{layer}{c}")
                nc.scalar.activation(tcn, ct_new[:, hs], AF.Tanh, scale=0.5)
                sig[c] = tcn
            for c in range(2):
                o0 = c * 512
                so = tall[:, o0 + 384 : o0 + 512]
                hs = slice(c * 128, (c + 1) * 128)
                nc.vector.scalar_tensor_tensor(Hnew[:, hs], so, 1.0, sig[c], OP.add, OP.mult)
                tp = tpsum.tile([128, 128], F16, tag="hT", name="tp")
                nc.tensor.transpose(tp, Hnew[:, hs], ident)
                nc.vector.tensor_copy(hT_new[:, hs], tp)
            ct[layer] = ct_new
            hT[layer] = hT_new

        # prologue: open gates0 accumulation for step 0 with the hh matmuls
        g0 = gpsum.tile([BLOC, G4], F32, tag="g0", name="g0")
        emit_hh(g0, 0, True)

        for t in range(steps):
            xsb = xsb_pp[(t + 1) % 2]  # tile being written this step (for step t+1)

            # close gates0: input matmuls (x'' includes tanh/lse/bias columns)
            for c in range(2):
                cs = _chunk_cols(c)
                nc.tensor.matmul(g0[:, cs], xT, w_ih0[:, cs], start=False, stop=True)

            # open gates1 with its hh + bias matmuls (ready at step start)
            g1 = gpsum.tile([BLOC, G4], F32, tag="g1", name="g1")
            emit_hh(g1, 1, False)
            for c in range(2):
                cs = _chunk_cols(c)
                nc.tensor.matmul(g1[:, cs], ones1, b1row[:, cs], start=False, stop=False)

            # layer-0 cell
            emit_cell(g0, 0, t)

            # close gates1 k-tile-major; next-step g0 hh fillers strictly AFTER
            # the ring-critical ih matmuls so they never win scheduler ties
            if t != steps - 1:
                g0n = gpsum.tile([BLOC, G4], F32, tag="g0", name="g0")
            else:
                g0n = None
            for c in range(2):
                cs = _chunk_cols(c)
                nc.tensor.matmul(g1[:, cs], hT[0][:, 0:128], w_ih1[0][:, cs], start=False, stop=False)
            for c in range(2):
                cs = _chunk_cols(c)
                nc.tensor.matmul(g1[:, cs], hT[0][:, 128:256], w_ih1[1][:, cs], start=False, stop=True)
            if g0n is not None:
                for c in range(2):
                    cs = _chunk_cols(c)
                    nc.tensor.matmul(g0n[:, cs], hT[0][:, 0:128], w_hh0[0][:, cs], start=True, stop=False)
                for c in range(2):
                    cs = _chunk_cols(c)
                    nc.tensor.matmul(g0n[:, cs], hT[0][:, 128:256], w_hh0[1][:, cs], start=False, stop=False)

            # layer-1 cell
            emit_cell(g1, 1, t)

            # FC
            pp = ppsum.tile([BLOC, OUTPUT_SIZE], F32, tag="pp", name="pp")
            nc.tensor.matmul(pp, ones1, bfcrow, start=True, stop=False)
            nc.tensor.matmul(pp, hT[1][:, 0:128], w_fc[0], start=False, stop=False)
            nc.tensor.matmul(pp, hT[1][:, 128:256], w_fc[1], start=False, stop=True)

            # softmax pieces: s = sum(exp(p)), tp-col, lse via bit-hack+Newton
            e63 = work.tile([BLOC, 63], F32, tag="e63", name="e63")
            ssum = work.tile([BLOC, 1], F32, tag="ssum", name="ssum")
            nc.scalar.activation(e63, pp[:, 0:63], AF.Exp, accum_out=ssum)
            conv = work.tile([BLOC, 1], F32, tag="conv", name="conv")
            ny0 = work.tile([BLOC, 1], F32, tag="ny0", name="ny0")
            en = work.tile([BLOC, 1], F32, tag="en", name="en")
            u = work.tile([BLOC, 1], F32, tag="u", name="u")
            nlse = work.tile([BLOC, 1], F32, tag="nlse", name="nlse")
            nc.vector.tensor_copy(conv, ssum.bitcast(I32))
            nc.vector.tensor_scalar(ny0, conv, -LN_A, -LN_B, OP.mult, OP.add)
            nc.scalar.activation(en, ny0, AF.Exp)
            nc.scalar.activation(xsb[:, 63:64], pp[:, 63:64], AF.Tanh, scale=0.5)
            nc.vector.tensor_copy(xsb[:, 0:63], pp[:, 0:63])
            nc.vector.tensor_tensor(u, ssum, en, OP.mult)
            nc.vector.scalar_tensor_tensor(nlse, ny0, 1.0, u, OP.add, OP.subtract)

            # next-step input columns + transpose (ring-critical)
            nc.vector.tensor_copy(xsb[:, 64:65], nlse)
            if t != steps - 1:
                xT_new = statep.tile([INPUT_SIZE + 2, BLOC], F16, tag="xT", name="xT")
                tpx = tpsum.tile([INPUT_SIZE + 2, BLOC], F16, tag="hT", name="tpx")
                nc.tensor.transpose(tpx, xsb, ident)
                nc.vector.tensor_copy(xT_new, tpx)
                xT = xT_new

            # raw per-step output snapshot: host derives log_softmax + dur
            nc.sync.dma_start(
                d_outs[:, (t % 16 if timing_ring else t), :], xsb[:, :]
            )
            g0 = g0n

        # ---------------- final states -----------------------------------
        nc.sync.dma_start(d_hf[0], Hnew_last[0])
        nc.sync.dma_start(d_hf[1], Hnew_last[1])
        nc.sync.dma_start(d_cf[0], ct[0])
        nc.sync.dma_start(d_cf[1], ct[1])

    return nc


def _legalize_waits(nc):
    """walrus accepts at most one sync-wait per compute/DMA instruction.
    Hoist excess waits onto same-engine NoOps inserted just before."""
    uid = 0
    for bb in nc.m.functions[0].blocks:
        il = bb.instructions
        if not any(
            getattr(i, "sync_info", None)
            and i.sync_info.on_wait
            and len(i.sync_info.on_wait) > 1
            for i in il
        ):
            continue
        new = []
        for inst in il:
            si = getattr(inst, "sync_info", None)
            if si and si.on_wait and len(si.on_wait) > 1:
                waits = list(si.on_wait)
                for w in waits[:-1]:
                    nop = mybir.InstEventSemaphore(name=f"waitnop-{uid}", ins=[], outs=[])
                    uid += 1
                    nop.engine = inst.engine
                    nop.sync_info = mybir.SyncInfo(on_wait=[w], on_update=[])
                    new.append(nop)
                inst.sync_info = mybir.SyncInfo(
                    on_wait=[waits[-1]], on_update=si.on_update
                )
            new.append(inst)
        bb.instructions = new


def prep_inputs(inputs, steps=MAX_LENGTH):
    """Host-side prep: returns per-core in_maps for run_bass_kernel_spmd."""
    f = np.asarray
    Wih0 = f(inputs["Wih0"], np.float32)
    Whh0 = f(inputs["Whh0"], np.float32)
    Wih1 = f(inputs["Wih1"], np.float32)
    Whh1 = f(inputs["Whh1"], np.float32)
    Wfc = f(inputs["Wfc"], np.float32)
    bih0 = f(inputs["bih0"], np.float32)
    bhh0 = f(inputs["bhh0"], np.float32)
    bih1 = f(inputs["bih1"], np.float32)
    bhh1 = f(inputs["bhh1"], np.float32)
    bfc = f(inputs["bfc"], np.float32)
    h0 = f(inputs["h0"], np.float32)
    c0 = f(inputs["c0"], np.float32)

    gsp = _GS_PERM[:, None]
    V0 = (Wih0 * _GS[:, None])[_PERM]  # [1024, 64] permuted+scaled
    b0 = ((bih0 + bhh0) * _GS)[_PERM]

    # x'' columns: [p(63) | tanh(p63/2) | -lse | 1]
    w_ih0 = np.zeros((INPUT_SIZE + 2, G4), np.float16)
    w_ih0[0:63] = V0[:, 0:63].T
    w_ih0[63] = 0.5 * V0[:, 63]
    w_ih0[64] = V0[:, 0:63].sum(1)
    w_ih0[65] = b0 + 0.5 * V0[:, 63]

    whh0 = (Whh0[_PERM] * gsp * 0.5).T.astype(np.float16)  # [256, 1024]
    wih1 = (Wih1[_PERM] * gsp * 0.5).T.astype(np.float16)
    whh1 = (Whh1[_PERM] * gsp * 0.5).T.astype(np.float16)
    wfc = (Wfc * 0.5).T.astype(np.float16)  # [256, 64]
    b1row = ((bih1 + bhh1) * _GS)[_PERM][None].astype(np.float16)
    bfcrow = bfc[None].astype(np.float16)

    xt_init = np.zeros((INPUT_SIZE + 2, BLOC), np.float16)
    xt_init[61] = 1.0   # SOS one-hot
    xt_init[63] = -1.0  # tanh col: 0.5*(-1)+0.5 = 0 = x0[63]
    xt_init[65] = 1.0   # const-1 bias row

    shared = dict(
        w_ih0=w_ih0,
        w_hh0a=whh0[0:128].copy(), w_hh0b=whh0[128:256].copy(),
        w_ih1a=wih1[0:128].copy(), w_ih1b=wih1[128:256].copy(),
        w_hh1a=whh1[0:128].copy(), w_hh1b=whh1[128:256].copy(),
        w_fca=wfc[0:128].copy(), w_fcb=wfc[128:256].copy(),
        b1row=b1row, bfcrow=bfcrow,
        xt_init=xt_init,
    )

    in_maps = []
    for i in range(NCORES):
        bsl = slice(i * BLOC, (i + 1) * BLOC)
        H0 = 2.0 * h0[0][bsl]  # [128, 256]
        H1 = 2.0 * h0[1][bsl]
        m = dict(shared)
        m["h0t_init"] = np.concatenate([H0[:, 0:128].T, H0[:, 128:256].T], 1).astype(np.float16)
        m["h1t_init"] = np.concatenate([H1[:, 0:128].T, H1[:, 128:256].T], 1).astype(np.float16)
        m["ct0_init"] = (2.0 * c0[0][bsl]).astype(np.float32)
        m["ct1_init"] = (2.0 * c0[1][bsl]).astype(np.float32)
        in_maps.append(m)
    return in_maps


_BUILD_CACHE = {}


def get_built(steps=MAX_LENGTH):
    if steps not in _BUILD_CACHE:
        _BUILD_CACHE[steps] = build_bass(steps)
    return _BUILD_CACHE[steps]


def run(inputs, steps=MAX_LENGTH, trace=False, **kw):
    nc = get_built(steps)
    if not getattr(nc, "_waits_legalized", False):
        _legalize_waits(nc)
        nc._waits_legalized = True
    in_maps = prep_inputs(inputs, steps)
    res = run_bass_kernel_spmd(nc, in_maps, core_ids=list(range(NCORES)), trace=trace, **kw)
    raw = np.concatenate([r["outs"] for r in res.results], 0).astype(np.float32)
    # raw[:, t] = [p(63) | tanh(p63/2) | -lse | 1]
    outputs = np.empty((BATCH, MAX_LENGTH, OUTPUT_SIZE), np.float32)
    outputs[:, :, 0:63] = raw[:, :, 0:63] + raw[:, :, 64:65]
    outputs[:, :, 63] = 0.5 * raw[:, :, 63] + 0.5
    h_f = np.stack(
        [
            np.concatenate([r["hf"][0].astype(np.float32) for r in res.results], 0),
            np.concatenate([r["hf"][1].astype(np.float32) for r in res.results], 0),
        ]
    ) * 0.5
    c_f = np.stack(
        [
            np.concatenate([r["cf"][0] for r in res.results], 0),
            np.concatenate([r["cf"][1] for r in res.results], 0),
        ]
    ) * 0.5
    return (outputs, h_f, c_f), res


def kernel(**inputs):
    (outputs, h_f, c_f), _ = run(inputs)
    return outputs, h_f, c_f


# revision 35
# speedup vs baseline: 1.0557x; 1.0226x over previous
"""Trainium2 Bass kernel for a 2-layer LSTM decoder (autoregressive, 512 steps).

Strategy (data-parallel over batch):
  - BATCH=1024 split as 128 rows per core across 8 cores; the 128 batch rows
    sit on the SBUF/PSUM partition dimension.
  - Matmuls are state-stationary fp16: out[batch, gates] = lhsT.T @ rhs with
    lhsT = transposed state tiles [K=h/x dims, M=128 batch] and rhs = host-
    pre-transposed weights [K, 4H] streamed N=512 per PSUM bank.
  - Single ACT table set (exp_and_others): sigmoid(x) computed as
    0.5*tanh(x/2)+0.5 with the 0.5 gate-scales folded into the weights and the
    (s+1) affines folded into fused scalar_tensor_tensor ops; h-state kept
    doubled (H=2h) with h-side weights pre-halved so no extra scaling ops.
  - log-sum-exp for log_softmax: exp with ACT accum_out gives s = sum(exp(p));
    ln(s) via exponent bit-hack initial guess + one Newton step (uses exp,
    stays in the same ACT table set).
  - The -lse and sigmoid(p63) contributions to the next step's input are
    applied as a K=2 rank-1 matmul so they stay off the critical path.
"""

import math
import sys

import numpy as np

if "/opt/trn_rl_repo" not in sys.path:
    sys.path.insert(0, "/opt/trn_rl_repo")

import concourse.bass as bass
import concourse.mybir as mybir
import concourse.tile as tile
from concourse.bass_utils import run_bass_kernel_spmd
from concourse.masks import make_identity

F16 = mybir.dt.float16
F32 = mybir.dt.float32
I32 = mybir.dt.int32
AF = mybir.ActivationFunctionType
OP = mybir.AluOpType

INPUT_SIZE = 64
HIDDEN = 256
OUTPUT_SIZE = 64
MAX_LENGTH = 512
BATCH = 1024
NCORES = 8
BLOC = BATCH // NCORES  # 128
G4 = 4 * HIDDEN  # 1024

# ln(x) ~= A*float(bitcast_i32(x)) + B, then one Newton step y += x*exp(-y)-1
LN2 = 0.6931471805599453
LN_A = LN2 / (1 << 23)
LN_B = LN2 * (0.0430357 - 127.0)

# gate permutation: torch order rows [i f g o] each 256 -> chunked layout
# [i_a f_a g_a o_a | i_b f_b g_b o_b] with a = h-dims 0:128, b = 128:256
_PERM = np.concatenate(
    [
        np.r_[0:128], np.r_[512:640], np.r_[256:384], np.r_[768:896],
        np.r_[128:256], np.r_[640:768], np.r_[384:512], np.r_[896:1024],
    ]
)
# gate scale: 0.5 for i,f,o (sigmoid-via-tanh), 1.0 for g (plain tanh)
_GS = np.ones(G4, np.float32)
_GS[0:256] = 0.5
_GS[256:512] = 0.5
_GS[768:1024] = 0.5
_GS_PERM = _GS[_PERM]


def _chunk_cols(c):
    return slice(c * 512, (c + 1) * 512)


def build_bass(steps=MAX_LENGTH, timing_ring=False):
    """Build the single-core Bass program (identical across cores).

    timing_ring=True shrinks the outputs tensor to a 16-step ring so timing
    runs avoid the large host<->device transfer (outputs then invalid).
    """
    from contextlib import ExitStack

    nc = bass.Bass("TRN2", target_bir_lowering=False, debug=False)

    # --- DRAM I/O ----------------------------------------------------------
    d_ih0 = nc.dram_tensor("w_ih0", [INPUT_SIZE + 2, G4], F16, kind="ExternalInput").ap()
    d_hh0a = nc.dram_tensor("w_hh0a", [128, G4], F16, kind="ExternalInput").ap()
    d_hh0b = nc.dram_tensor("w_hh0b", [128, G4], F16, kind="ExternalInput").ap()
    d_ih1a = nc.dram_tensor("w_ih1a", [128, G4], F16, kind="ExternalInput").ap()
    d_ih1b = nc.dram_tensor("w_ih1b", [128, G4], F16, kind="ExternalInput").ap()
    d_hh1a = nc.dram_tensor("w_hh1a", [128, G4], F16, kind="ExternalInput").ap()
    d_hh1b = nc.dram_tensor("w_hh1b", [128, G4], F16, kind="ExternalInput").ap()
    d_fca = nc.dram_tensor("w_fca", [128, OUTPUT_SIZE], F16, kind="ExternalInput").ap()
    d_fcb = nc.dram_tensor("w_fcb", [128, OUTPUT_SIZE], F16, kind="ExternalInput").ap()
    d_b1 = nc.dram_tensor("b1row", [1, G4], F16, kind="ExternalInput").ap()
    d_bfc = nc.dram_tensor("bfcrow", [1, OUTPUT_SIZE], F16, kind="ExternalInput").ap()
    d_h0t = nc.dram_tensor("h0t_init", [128, HIDDEN], F16, kind="ExternalInput").ap()
    d_h1t = nc.dram_tensor("h1t_init", [128, HIDDEN], F16, kind="ExternalInput").ap()
    d_ct0 = nc.dram_tensor("ct0_init", [BLOC, HIDDEN], F32, kind="ExternalInput").ap()
    d_ct1 = nc.dram_tensor("ct1_init", [BLOC, HIDDEN], F32, kind="ExternalInput").ap()
    d_xt = nc.dram_tensor("xt_init", [INPUT_SIZE + 2, BLOC], F16, kind="ExternalInput").ap()

    d_outs = nc.dram_tensor(
        "outs", [BLOC, 16 if timing_ring else steps, INPUT_SIZE + 2], F16,
        kind="ExternalOutput",
    ).ap()
    d_hf = nc.dram_tensor("hf", [2, BLOC, HIDDEN], F16, kind="ExternalOutput").ap()
    d_cf = nc.dram_tensor("cf", [2, BLOC, HIDDEN], F32, kind="ExternalOutput").ap()

    with ExitStack() as ctx:
        tc = ctx.enter_context(tile.TileContext(nc))
        singles = ctx.enter_context(tc.tile_pool(name="singles", bufs=1))
        work = ctx.enter_context(tc.tile_pool(name="work", bufs=3))
        gpsum = ctx.enter_context(tc.tile_pool(name="gpsum", bufs=1, space="PSUM"))
        ppsum = ctx.enter_context(tc.tile_pool(name="ppsum", bufs=1, space="PSUM"))
        tpsum = ctx.enter_context(tc.tile_pool(name="tpsum", bufs=3, space="PSUM"))

        # --- persistent SBUF ----------------------------------------------
        w_ih0 = singles.tile([INPUT_SIZE + 2, G4], F16)
        w_hh0 = [singles.tile([128, G4], F16, tag=f"whh0{c}", name=f"whh0{c}") for c in range(2)]
        w_ih1 = [singles.tile([128, G4], F16, tag=f"wih1{c}", name=f"wih1{c}") for c in range(2)]
        w_hh1 = [singles.tile([128, G4], F16, tag=f"whh1{c}", name=f"whh1{c}") for c in range(2)]
        w_fc = [singles.tile([128, OUTPUT_SIZE], F16, tag=f"wfc{c}", name=f"wfc{c}") for c in range(2)]
        b1row = singles.tile([1, G4], F16)
        bfcrow = singles.tile([1, OUTPUT_SIZE], F16)
        ident = singles.tile([128, 128], F16)
        ones1 = singles.tile([1, 128], F16)
        xsb_pp = [
            singles.tile([128, INPUT_SIZE + 2], F16, tag=f"xsb{i}", name=f"xsb{i}")
            for i in range(2)
        ]
        xT = singles.tile([INPUT_SIZE + 2, BLOC], F16)
        h0T = singles.tile([128, HIDDEN], F16)
        h1T = singles.tile([128, HIDDEN], F16)
        statep = ctx.enter_context(tc.tile_pool(name="statep", bufs=3))

        nc.sync.dma_start(w_ih0, d_ih0)
        nc.sync.dma_start(w_hh0[0], d_hh0a)
        nc.sync.dma_start(w_hh0[1], d_hh0b)
        nc.sync.dma_start(w_ih1[0], d_ih1a)
        nc.sync.dma_start(w_ih1[1], d_ih1b)
        nc.sync.dma_start(w_hh1[0], d_hh1a)
        nc.sync.dma_start(w_hh1[1], d_hh1b)
        nc.sync.dma_start(w_fc[0], d_fca)
        nc.sync.dma_start(w_fc[1], d_fcb)
        nc.sync.dma_start(b1row, d_b1)
        nc.sync.dma_start(bfcrow, d_bfc)
        nc.sync.dma_start(h0T, d_h0t)
        nc.sync.dma_start(h1T, d_h1t)
        nc.sync.dma_start(xT, d_xt)

        make_identity(nc, ident)
        nc.vector.memset(ones1, 1.0)
        for i in range(2):
            nc.vector.memset(xsb_pp[i][:, 0:66], 0.0)
            nc.vector.memset(xsb_pp[i][:, 65:66], 1.0)
            nc.vector.memset(xsb_pp[i][:, 61:62], 1.0)  # step-0 one-hot (overwritten later)
            nc.vector.memset(xsb_pp[i][:, 63:64], -1.0)

        ct = [singles.tile([BLOC, HIDDEN], F32, tag=f"ct_init{l}", name=f"ct_init{l}") for l in range(2)]
        nc.sync.dma_start(ct[0], d_ct0)
        nc.sync.dma_start(ct[1], d_ct1)

        hT = [h0T, h1T]
        w_hh = [w_hh0, w_hh1]
        Hnew_last = [None, None]

        def emit_hh(g, layer, first, opened=False):
            """hh matmuls; open each chunk's accum group unless already opened."""
            for c in range(2):
                cs = _chunk_cols(c)
                nc.tensor.matmul(g[:, cs], hT[layer][:, 0:128], w_hh[layer][0][:, cs], start=not opened, stop=False)
                nc.tensor.matmul(g[:, cs], hT[layer][:, 128:256], w_hh[layer][1][:, cs], start=False, stop=False)

        def emit_cell(g, layer, t):
            """tanh + cell update for one layer; returns nothing (updates hT/ct)."""
            tall = work.tile([BLOC, G4], F16, tag=f"tall{layer}", name=f"tall{layer}")
            ct_new = work.tile([BLOC, HIDDEN], F32, tag=f"ct{layer}", name=f"ct{layer}")
            Hnew = work.tile([BLOC, HIDDEN], F16, tag=f"H{layer}", name=f"H{layer}")
            hT_new = statep.tile([128, HIDDEN], F16, tag=f"hT{layer}", name=f"hT{layer}")
            Hnew_last[layer] = Hnew
            sig = {}
            # ring-critical tanh covers only [i g f]; the o-gate tanh is an
            # ACT-queue filler (H needs it much later), keeping the queue
            # clear for tanh(c)
            for c in range(2):
                o0 = c * 512
                nc.scalar.activation(tall[:, o0 : o0 + 384], g[:, o0 : o0 + 384], AF.Tanh)
            for c in range(2):
                o0 = c * 512
                nc.scalar.activation(tall[:, o0 + 384 : o0 + 512], g[:, o0 + 384 : o0 + 512], AF.Tanh)
            for c in range(2):
                o0 = c * 512
                si = tall[:, o0 : o0 + 128]
                tg = tall[:, o0 + 128 : o0 + 256]
                sf = tall[:, o0 + 256 : o0 + 384]
                hs = slice(c * 128, (c + 1) * 128)
                A = work.tile([BLOC, 128], F16, tag=f"A{layer}{c}", name=f"A{layer}{c}")
                B = work.tile([BLOC, 128], F32, tag=f"B{layer}{c}", name=f"B{layer}{c}")
                # state is D = 2c:  A = 2*sig_i*tanh(g);  B = 2*sig_f*D
                # D_new = 2*c_new = 0.5*B + A;  tanh(c_new) = tanh(0.5*D_new)
                nc.vector.scalar_tensor_tensor(A, si, 1.0, tg, OP.add, OP.mult)
                nc.vector.scalar_tensor_tensor(B, sf, 1.0, ct[layer][:, hs], OP.add, OP.mult)
                nc.vector.scalar_tensor_tensor(ct_new[:, hs], B, 0.5, A, OP.mult, OP.add)
            for c in range(2):
                hs = slice(c * 128, (c + 1) * 128)
                tcn = work.tile([BLOC, 128], F16, tag=f"tc{layer}{c}", name=f"tc{layer}{c}")
                nc.scalar.activation(tcn, ct_new[:, hs], AF.Tanh, scale=0.5)
                sig[c] = tcn
            for c in range(2):
                o0 = c * 512
                so = tall[:, o0 + 384 : o0 + 512]
                hs = slice(c * 128, (c + 1) * 128)
                nc.vector.scalar_tensor_tensor(Hnew[:, hs], so, 1.0, sig[c], OP.add, OP.mult)
                tp = tpsum.tile([128, 128], F16, tag="hT", name="tp")
                nc.tensor.transpose(tp, Hnew[:, hs], ident)
                nc.vector.tensor_copy(hT_new[:, hs], tp)
            ct[layer] = ct_new
            hT[layer] = hT_new

        # prologue: open gates0 accumulation for step 0 with the hh matmuls
        g0 = gpsum.tile([BLOC, G4], F32, tag="g0", name="g0")
        emit_hh(g0, 0, True)

        for t in range(steps):
            xsb = xsb_pp[(t + 1) % 2]  # tile being written this step (for step t+1)

            # close gates0: input matmuls (x'' includes tanh/lse/bias columns)
            for c in range(2):
                cs = _chunk_cols(c)
                nc.tensor.matmul(g0[:, cs], xT, w_ih0[:, cs], start=False, stop=True)

            # open gates1 with its hh + bias matmuls (ready at step start)
            g1 = gpsum.tile([BLOC, G4], F32, tag="g1", name="g1")
            for c in range(2):
                cs = _chunk_cols(c)
                nc.tensor.matmul(g1[:, cs], ones1, b1row[:, cs], start=True, stop=False)
            emit_hh(g1, 1, False, opened=True)

            # layer-0 cell
            emit_cell(g0, 0, t)

            # close gates1 k-tile-major; next-step g0 hh fillers strictly AFTER
            # the ring-critical ih matmuls so they never win scheduler ties
            if t != steps - 1:
                g0n = gpsum.tile([BLOC, G4], F32, tag="g0", name="g0")
            else:
                g0n = None
            for c in range(2):
                cs = _chunk_cols(c)
                nc.tensor.matmul(g1[:, cs], hT[0][:, 0:128], w_ih1[0][:, cs], start=False, stop=False)
            for c in range(2):
                cs = _chunk_cols(c)
                nc.tensor.matmul(g1[:, cs], hT[0][:, 128:256], w_ih1[1][:, cs], start=False, stop=True)
            if g0n is not None:
                for c in range(2):
                    cs = _chunk_cols(c)
                    nc.tensor.matmul(g0n[:, cs], hT[0][:, 0:128], w_hh0[0][:, cs], start=True, stop=False)
                for c in range(2):
                    cs = _chunk_cols(c)
                    nc.tensor.matmul(g0n[:, cs], hT[0][:, 128:256], w_hh0[1][:, cs], start=False, stop=False)

            # layer-1 cell
            emit_cell(g1, 1, t)

            # FC
            pp = ppsum.tile([BLOC, OUTPUT_SIZE], F32, tag="pp", name="pp")
            nc.tensor.matmul(pp, ones1, bfcrow, start=True, stop=False)
            nc.tensor.matmul(pp, hT[1][:, 0:128], w_fc[0], start=False, stop=False)
            nc.tensor.matmul(pp, hT[1][:, 128:256], w_fc[1], start=False, stop=True)

            # softmax pieces: s = sum(exp(p)), tp-col, lse via bit-hack+Newton
            e63 = work.tile([BLOC, 63], F32, tag="e63", name="e63")
            ssum = work.tile([BLOC, 1], F32, tag="ssum", name="ssum")
            nc.scalar.activation(e63, pp[:, 0:63], AF.Exp, accum_out=ssum)
            conv = work.tile([BLOC, 1], F32, tag="conv", name="conv")
            ny0 = work.tile([BLOC, 1], F32, tag="ny0", name="ny0")
            en = work.tile([BLOC, 1], F32, tag="en", name="en")
            u = work.tile([BLOC, 1], F32, tag="u", name="u")
            nlse = work.tile([BLOC, 1], F32, tag="nlse", name="nlse")
            nc.vector.tensor_copy(conv, ssum.bitcast(I32))
            nc.vector.tensor_scalar(ny0, conv, -LN_A, -LN_B, OP.mult, OP.add)
            nc.scalar.activation(en, ny0, AF.Exp)
            nc.scalar.activation(xsb[:, 63:64], pp[:, 63:64], AF.Tanh, scale=0.5)
            nc.vector.tensor_copy(xsb[:, 0:63], pp[:, 0:63])
            nc.vector.tensor_tensor(u, ssum, en, OP.mult)
            nc.vector.scalar_tensor_tensor(nlse, ny0, 1.0, u, OP.add, OP.subtract)

            # next-step input columns + transpose (ring-critical)
            nc.vector.tensor_copy(xsb[:, 64:65], nlse)
            if t != steps - 1:
                xT_new = statep.tile([INPUT_SIZE + 2, BLOC], F16, tag="xT", name="xT")
                tpx = tpsum.tile([INPUT_SIZE + 2, BLOC], F16, tag="hT", name="tpx")
                nc.tensor.transpose(tpx, xsb, ident)
                nc.vector.tensor_copy(xT_new, tpx)
                xT = xT_new

            # raw per-step output snapshot: host derives log_softmax + dur
            nc.sync.dma_start(
                d_outs[:, (t % 16 if timing_ring else t), :], xsb[:, :]
            )
            g0 = g0n

        # ---------------- final states -----------------------------------
        nc.sync.dma_start(d_hf[0], Hnew_last[0])
        nc.sync.dma_start(d_hf[1], Hnew_last[1])
        nc.sync.dma_start(d_cf[0], ct[0])
        nc.sync.dma_start(d_cf[1], ct[1])

    return nc


def _legalize_waits(nc):
    """walrus accepts at most one sync-wait per compute/DMA instruction.
    Hoist excess waits onto same-engine NoOps inserted just before."""
    uid = 0
    for bb in nc.m.functions[0].blocks:
        il = bb.instructions
        if not any(
            getattr(i, "sync_info", None)
            and i.sync_info.on_wait
            and len(i.sync_info.on_wait) > 1
            for i in il
        ):
            continue
        new = []
        for inst in il:
            si = getattr(inst, "sync_info", None)
            if si and si.on_wait and len(si.on_wait) > 1:
                waits = list(si.on_wait)
                for w in waits[:-1]:
                    nop = mybir.InstEventSemaphore(name=f"waitnop-{uid}", ins=[], outs=[])
                    uid += 1
                    nop.engine = inst.engine
                    nop.sync_info = mybir.SyncInfo(on_wait=[w], on_update=[])
                    new.append(nop)
                inst.sync_info = mybir.SyncInfo(
                    on_wait=[waits[-1]], on_update=si.on_update
                )
            new.append(inst)
        bb.instructions = new


def prep_inputs(inputs, steps=MAX_LENGTH):
    """Host-side prep: returns per-core in_maps for run_bass_kernel_spmd."""
    f = np.asarray
    Wih0 = f(inputs["Wih0"], np.float32)
    Whh0 = f(inputs["Whh0"], np.float32)
    Wih1 = f(inputs["Wih1"], np.float32)
    Whh1 = f(inputs["Whh1"], np.float32)
    Wfc = f(inputs["Wfc"], np.float32)
    bih0 = f(inputs["bih0"], np.float32)
    bhh0 = f(inputs["bhh0"], np.float32)
    bih1 = f(inputs["bih1"], np.float32)
    bhh1 = f(inputs["bhh1"], np.float32)
    bfc = f(inputs["bfc"], np.float32)
    h0 = f(inputs["h0"], np.float32)
    c0 = f(inputs["c0"], np.float32)

    gsp = _GS_PERM[:, None]
    V0 = (Wih0 * _GS[:, None])[_PERM]  # [1024, 64] permuted+scaled
    b0 = ((bih0 + bhh0) * _GS)[_PERM]

    # x'' columns: [p(63) | tanh(p63/2) | -lse | 1]
    w_ih0 = np.zeros((INPUT_SIZE + 2, G4), np.float16)
    w_ih0[0:63] = V0[:, 0:63].T
    w_ih0[63] = 0.5 * V0[:, 63]
    w_ih0[64] = V0[:, 0:63].sum(1)
    w_ih0[65] = b0 + 0.5 * V0[:, 63]

    whh0 = (Whh0[_PERM] * gsp * 0.5).T.astype(np.float16)  # [256, 1024]
    wih1 = (Wih1[_PERM] * gsp * 0.5).T.astype(np.float16)
    whh1 = (Whh1[_PERM] * gsp * 0.5).T.astype(np.float16)
    wfc = (Wfc * 0.5).T.astype(np.float16)  # [256, 64]
    b1row = ((bih1 + bhh1) * _GS)[_PERM][None].astype(np.float16)
    bfcrow = bfc[None].astype(np.float16)

    xt_init = np.zeros((INPUT_SIZE + 2, BLOC), np.float16)
    xt_init[61] = 1.0   # SOS one-hot
    xt_init[63] = -1.0  # tanh col: 0.5*(-1)+0.5 = 0 = x0[63]
    xt_init[65] = 1.0   # const-1 bias row

    shared = dict(
        w_ih0=w_ih0,
        w_hh0a=whh0[0:128].copy(), w_hh0b=whh0[128:256].copy(),
        w_ih1a=wih1[0:128].copy(), w_ih1b=wih1[128:256].copy(),
        w_hh1a=whh1[0:128].copy(), w_hh1b=whh1[128:256].copy(),
        w_fca=wfc[0:128].copy(), w_fcb=wfc[128:256].copy(),
        b1row=b1row, bfcrow=bfcrow,
        xt_init=xt_init,
    )

    in_maps = []
    for i in range(NCORES):
        bsl = slice(i * BLOC, (i + 1) * BLOC)
        H0 = 2.0 * h0[0][bsl]  # [128, 256]
        H1 = 2.0 * h0[1][bsl]
        m = dict(shared)
        m["h0t_init"] = np.concatenate([H0[:, 0:128].T, H0[:, 128:256].T], 1).astype(np.float16)
        m["h1t_init"] = np.concatenate([H1[:, 0:128].T, H1[:, 128:256].T], 1).astype(np.float16)
        m["ct0_init"] = (2.0 * c0[0][bsl]).astype(np.float32)
        m["ct1_init"] = (2.0 * c0[1][bsl]).astype(np.float32)
        in_maps.append(m)
    return in_maps


_BUILD_CACHE = {}


def get_built(steps=MAX_LENGTH):
    if steps not in _BUILD_CACHE:
        _BUILD_CACHE[steps] = build_bass(steps)
    return _BUILD_CACHE[steps]


def run(inputs, steps=MAX_LENGTH, trace=False, **kw):
    nc = get_built(steps)
    if not getattr(nc, "_waits_legalized", False):
        _legalize_waits(nc)
        nc._waits_legalized = True
    in_maps = prep_inputs(inputs, steps)
    res = run_bass_kernel_spmd(nc, in_maps, core_ids=list(range(NCORES)), trace=trace, **kw)
    raw = np.concatenate([r["outs"] for r in res.results], 0).astype(np.float32)
    # raw[:, t] = [p(63) | tanh(p63/2) | -lse | 1]
    outputs = np.empty((BATCH, MAX_LENGTH, OUTPUT_SIZE), np.float32)
    outputs[:, :, 0:63] = raw[:, :, 0:63] + raw[:, :, 64:65]
    outputs[:, :, 63] = 0.5 * raw[:, :, 63] + 0.5
    h_f = np.stack(
        [
            np.concatenate([r["hf"][0].astype(np.float32) for r in res.results], 0),
            np.concatenate([r["hf"][1].astype(np.float32) for r in res.results], 0),
        ]
    ) * 0.5
    c_f = np.stack(
        [
            np.concatenate([r["cf"][0] for r in res.results], 0),
            np.concatenate([r["cf"][1] for r in res.results], 0),
        ]
    ) * 0.5
    return (outputs, h_f, c_f), res


def kernel(**inputs):
    (outputs, h_f, c_f), _ = run(inputs)
    return outputs, h_f, c_f
